# revision 13
# baseline (speedup 1.0000x reference)
"""Multi-head attention (B=2, S=2048, H=2048, 16 heads, d=128) on 8 TRN2
NeuronCores.

Sharding: 2-way batch x 4-way head-group tensor parallel. Core j handles
batch j//4 and heads 4*(j%4)..4*(j%4)+3 (a 512-wide slice of the qkv
projection output dim / o_proj input dim). Each core returns a partial
o_proj output [S, H] in fp16; the host sums the 4 partials per batch and
adds an effective bias bo + wo@bv (softmax rows sum to 1, so the v bias
contributes a constant; bk cancels inside softmax and is dropped).

All fp16 matmul operands, fp32 PSUM accumulation. DRAM layouts keep
16KB-contiguous per-partition rows (~420GB/s DMA vs ~200 for 1KB rows);
x block 0 is split into 4 seq-quarter DMAs so the first v tile starts
~5us earlier. One interleaved PE stream:

  P1: v = x@wv.T and head 0's qT/kT projections, seq-blocked, while x
      streams in.
  P2: for h in 0..2: attention(h) stages with head h+1's qT/kT projection
      matmuls as per-stage PE filler (x re-streamed from HBM per head).
  P3: attention(h=3) with o_proj matmuls of completed q-blocks as filler.
  P4: leftover o_proj through all idle PSUM banks.

Attention per (head, q-block), scoresT layout [k, q]:
  scoresT = kT_h.T @ qT_h -> exp on ScalarE -> expT (fp16)
  VectorE pair-adds exp chunks; sums += ones128.T @ pair  (half-cost rowsum)
  ctxT += v_chunk.T @ expT ; ctxT_norm = ctxT * approx_recip(sums)
"""
import sys

if "/opt/trn_rl_repo" not in sys.path:
    sys.path.insert(0, "/opt/trn_rl_repo")

import numpy as np

HIDDEN = 2048
HEADS = 16
HEAD_DIM = 128
BATCH = 2
SEQ = 2048

N_CORES = 8
GROUPS = 4               # head groups (cores per batch)
GDIM = HIDDEN // GROUPS  # 512 dims per core
GHEADS = GDIM // HEAD_DIM  # 4 heads per core
KC = HIDDEN // 128       # 16 contraction chunks
SB = 4                   # seq blocks of 512
QB = SEQ // 512          # 4 q-blocks in attention
MT = SEQ // 128          # 16 seq tiles of 128

_CACHE = {}


def _build():
    import concourse.bacc as bacc
    import concourse.bass as bass
    import concourse.bass_isa as bass_isa
    import concourse.mybir as mybir
    import concourse.tile as tile

    fp16 = mybir.dt.float16
    fp32 = mybir.dt.float32
    AF = mybir.ActivationFunctionType

    nc = bacc.Bacc("TRN2", target_bir_lowering=False, debug=False,
                   num_devices=N_CORES)

    # xt[sb, t, p, c*128+s'] = x[b, sb*512+t*128+s', c*128+p]
    xT = nc.dram_tensor("xt", [SB, 4, 128, KC * 128], fp16, kind="ExternalInput").ap()
    # wqt[h, p, c*128+m] = wq_scaled[h*128+m, c*128+p]
    wqT = nc.dram_tensor("wqt", [GHEADS, 128, KC * 128], fp16, kind="ExternalInput").ap()
    wkT = nc.dram_tensor("wkt", [GHEADS, 128, KC * 128], fp16, kind="ExternalInput").ap()
    # wvt[p, c*512+d] = wv[d, c*128+p]
    wvT = nc.dram_tensor("wvt", [128, KC * 512], fp16, kind="ExternalInput").ap()
    # wot[p, hh*2048+oc] = wo[oc, hh*128+p]
    woT = nc.dram_tensor("wot", [128, GHEADS * HIDDEN], fp16, kind="ExternalInput").ap()
    bq = nc.dram_tensor("bq", [GDIM], fp32, kind="ExternalInput").ap()
    # out[mq, p, oc] = partial[mq*128+p, oc], fp16
    out = nc.dram_tensor("out", [MT, 128, HIDDEN], fp16, kind="ExternalOutput").ap()

    with tile.TileContext(nc) as tc:
        with (
            tc.tile_pool(name="xp", bufs=2) as xp,        # 16KB x-block slots
            tc.tile_pool(name="wqk", bufs=2) as wqk,
            tc.tile_pool(name="wvo", bufs=2) as wvo,
            tc.tile_pool(name="res", bufs=1) as res,
            tc.tile_pool(name="ebp", bufs=1) as ebp,
            tc.tile_pool(name="epp", bufs=2) as epp,
            tc.tile_pool(name="small", bufs=1) as small,
            tc.tile_pool(name="rec", bufs=2) as rec,
            tc.tile_pool(name="sump", bufs=1) as sump,
            tc.tile_pool(name="outp", bufs=2) as outp,
            tc.tile_pool(name="ps_a", bufs=2, space=bass.MemorySpace.PSUM) as ps_a,
            tc.tile_pool(name="ps_sc", bufs=2, space=bass.MemorySpace.PSUM) as ps_sc,
            tc.tile_pool(name="ps_sum", bufs=2, space=bass.MemorySpace.PSUM) as ps_sum,
            tc.tile_pool(name="ps_ctx", bufs=2, space=bass.MemorySpace.PSUM) as ps_ctx,
        ):
            wq_sb = wqk.tile([128, GHEADS * KC * 128], fp16, tag="wqk", name="wq")
            wk_sb = wqk.tile([128, GHEADS * KC * 128], fp16, tag="wqk", name="wk")
            wv_sb = wvo.tile([128, KC * GDIM], fp16, tag="wvo", name="wv")

            qT_sb = res.tile([128, GHEADS * SEQ], fp16, tag="qT")
            kT_sb = res.tile([128, GHEADS * SEQ], fp16, tag="kT")
            v_sb = res.tile([128, MT * GDIM], fp16, tag="v")
            ctx_sb = res.tile([128, GHEADS * SEQ], fp16, tag="ctx")

            eblk = ebp.tile([128, KC * 512], fp16, tag="eblk")

            bq_sb = small.tile([128, GHEADS], fp32, tag="bq")
            ones_sb = small.tile([128, 128], fp16, tag="ones")
            nc.vector.memset(ones_sb[:], 1.0)

            # ---------- P0: initial DMAs + HAM warmup ----------
            # wv + x0 first (v tiles gate P1); x0 split by seq-quarter so
            # v_tile(0,0) starts after ~1.5MB instead of 3MB.
            xv = {0: xp.tile([128, 4 * KC * 128], fp16, tag="xp", name="x0")}
            nc.sync.dma_start(wv_sb[:, 0:4 * GDIM], wvT[:, 0:4 * GDIM])
            for t in range(4):
                nc.sync.dma_start(
                    xv[0][:, t * KC * 128:(t + 1) * KC * 128], xT[0, t])
                if t < 3:
                    nc.sync.dma_start(
                        wv_sb[:, (t + 1) * 4 * GDIM:(t + 2) * 4 * GDIM],
                        wvT[:, (t + 1) * 4 * GDIM:(t + 2) * 4 * GDIM])
            nc.sync.dma_start(bq_sb[:], bq.rearrange("(m p) -> p m", p=128))
            nc.sync.dma_start(wq_sb[:, 0:KC * 128], wqT[0])
            nc.sync.dma_start(wk_sb[:, 0:KC * 128], wkT[0])

            warm = ps_a.tile([128, 512], fp32, tag="ps_a", name="warm")
            for _ in range(60):
                nc.tensor.matmul(warm[:, :128], ones_sb[:], ones_sb[:],
                                 start=True, stop=True)

            def warm_trickle(n):
                wt = ps_ctx.tile([128, 64], fp32, tag="ps_ctx", name="wt")
                for _ in range(n):
                    nc.tensor.matmul(wt[:], ones_sb[:], ones_sb[:, :64],
                                     start=True, stop=True)

            # ---------- helpers ----------
            TQ = KC * 128  # 2048 cols per seq-quarter in an x block

            def v_tile(xblk, st):
                """One [128 seq, 512 dims] v tile (seq tile st = sb*4+t)."""
                t = st % 4
                ps = ps_a.tile([128, 512], fp32, tag="ps_a")
                for c in range(KC):
                    nc.tensor.matmul(
                        ps[:],
                        xblk[:, t * TQ + c * 128: t * TQ + (c + 1) * 128],
                        wv_sb[:, c * GDIM:(c + 1) * GDIM],
                        start=(c == 0), stop=(c == KC - 1))
                nc.vector.tensor_copy(v_sb[:, st * GDIM:(st + 1) * GDIM], ps[:])

            def xmov(xblk, c):
                """Moving-operand AP for chunk c over a 512-seq block:
                [128, 4 quarters, 128] strided view."""
                return xblk[:].rearrange(
                    "p (t r) -> p t r", t=4)[:, :, c * 128:(c + 1) * 128]

            def qk_tile(xblk, w_sb, b_sb, dst, h, s0, nm):
                """One [128 dims, 512 seq] q/k projection tile + bias copy."""
                ps = ps_sc.tile([128, 512], fp32, tag="ps_sc", name=f"pp{nm}")
                hw = h * KC * 128
                for c in range(KC):
                    nc.tensor.matmul(
                        ps[:],
                        w_sb[:, hw + c * 128: hw + (c + 1) * 128],
                        xmov(xblk, c),
                        start=(c == 0), stop=(c == KC - 1))
                nc.scalar.activation(
                    dst[:, h * SEQ + s0: h * SEQ + s0 + 512],
                    ps[:], AF.Identity,
                    bias=(b_sb[:, h:h + 1] if b_sb is not None else 0.0))

            # ---------- P1: v projection + head-0 q/k projection ----------
            for sb in range(SB):
                for t in range(4):
                    v_tile(xv[sb], sb * 4 + t)
                    if sb == 0 or (sb == 1 and t < 2):
                        warm_trickle(6)
                    if t == 1 and sb + 1 < SB:
                        xv[sb + 1] = xp.tile([128, 4 * TQ], fp16, tag="xp",
                                             name=f"x{sb + 1}")
                        nc.sync.dma_start(
                            xv[sb + 1][:].rearrange("p (t r) -> p t r", t=4),
                            xT[sb + 1].rearrange("t p r -> p t r"))
                    if t == 3 and sb == 0:
                        for h in range(1, GHEADS):
                            nc.sync.dma_start(
                                wq_sb[:, h * KC * 128:(h + 1) * KC * 128],
                                wqT[h])
                            nc.sync.dma_start(
                                wk_sb[:, h * KC * 128:(h + 1) * KC * 128],
                                wkT[h])
                qk_tile(xv[sb], wq_sb, bq_sb, qT_sb, 0, sb * 512, f"q{sb}")
                qk_tile(xv[sb], wk_sb, None, kT_sb, 0, sb * 512, f"k{sb}")

            # ---------- P2/P3: attention windows with PE filler ----------
            state = {}
            pend = []

            def drain(bi, kp):
                h, qb, eblk_, ep, sums, ctxp = state[bi]
                for kc in (2 * kp, 2 * kp + 1):
                    nc.tensor.matmul(ctxp[:],
                                     v_sb[:, kc * GDIM + h * 128:
                                          kc * GDIM + (h + 1) * 128],
                                     eblk_[:, kc * 512:(kc + 1) * 512],
                                     start=(kc == 0), stop=(kc == KC - 1))
                if kp == KC // 2 - 1:
                    if h < GHEADS - 1:
                        # off the critical path: idle GpSimd does the rowsum
                        nc.gpsimd.partition_all_reduce(
                            sums[:], ep[:, 0:512], 128, bass_isa.ReduceOp.add)
                    else:
                        # o_proj filler needs ctx promptly: fast PE rowsum
                        nc.tensor.matmul(sums[:], ones_sb[:],
                                         ep[:, 0:512], start=True, stop=True)
                    finish(bi)

            def finish(bi):
                h, qb, eblk_, ep, sums, ctxp = state.pop(bi)
                q0 = qb * 512
                recip = rec.tile([128, 512], fp32, tag="recip")
                nc.vector.reciprocal_approx_fast(recip[:], sums[:])
                nc.vector.tensor_mul(ctx_sb[:, h * SEQ + q0: h * SEQ + q0 + 512],
                                     ctxp[:], recip[:])

            # filler generators -------------------------------------------
            def proj_filler(h):
                """Yield 128 single-MM closures projecting head h's qT/kT,
                with x re-streamed per seq block (xp slots cycle)."""
                xb = {}

                def load_x(sb):
                    t = xp.tile([128, 4 * TQ], fp16, tag="xp",
                                name=f"xh{h}_{sb}")
                    nc.sync.dma_start(
                        t[:].rearrange("p (t r) -> p t r", t=4),
                        xT[sb].rearrange("t p r -> p t r"))
                    return t

                xb[0] = load_x(0)
                hw = h * KC * 128
                for sb in range(SB):
                    if sb + 1 < SB:
                        xb[sb + 1] = load_x(sb + 1)
                    s0 = sb * 512
                    for w_sb, b_sb, dst, nm in ((wq_sb, bq_sb, qT_sb, "q"),
                                                (wk_sb, None, kT_sb, "k")):
                        ps = ps_a.tile([128, 512], fp32, tag="ps_a",
                                       name=f"p{nm}{h}_{sb}")
                        for c in range(KC):
                            def mm(c=c, ps=ps, w_sb=w_sb, b_sb=b_sb, dst=dst,
                                   sb=sb, s0=s0):
                                nc.tensor.matmul(
                                    ps[:],
                                    w_sb[:, hw + c * 128: hw + (c + 1) * 128],
                                    xmov(xb[sb], c),
                                    start=(c == 0), stop=(c == KC - 1))
                                if c == KC - 1:
                                    nc.scalar.activation(
                                        dst[:, h * SEQ + s0: h * SEQ + s0 + 512],
                                        ps[:], AF.Identity,
                                        bias=(b_sb[:, h:h + 1]
                                              if b_sb is not None else 0.0))
                            yield mm

            def oproj_filler(qb, deep=False):
                """Yield 64 single-MM closures for o_proj q-tiles of block qb
                (all heads' ctx for qb must be finished)."""
                for i, (mq, oc) in enumerate(
                        (mq, oc) for mq in range(qb * 4, qb * 4 + 4)
                        for oc in range(4)):
                    if deep and i % 3 == 1:
                        ps = ps_sum.tile([128, 512], fp32, tag="ps_sum",
                                         name=f"po{mq}_{oc}")
                    elif deep and i % 3 == 2:
                        ps = ps_ctx.tile([128, 512], fp32, tag="ps_ctx",
                                         name=f"po{mq}_{oc}")
                    else:
                        ps = ps_a.tile([128, 512], fp32, tag="ps_a",
                                       name=f"po{mq}_{oc}")
                    ostage = ostages[mq % 2]
                    for hh in range(GHEADS):
                        def mm(ps=ps, hh=hh, mq=mq, oc=oc, i=i, ostage=ostage):
                            nc.tensor.matmul(
                                ps[:],
                                ctx_sb[:, hh * SEQ + mq * 128:
                                       hh * SEQ + (mq + 1) * 128],
                                wo_sb[:, hh * HIDDEN + oc * 512:
                                      hh * HIDDEN + (oc + 1) * 512],
                                start=(hh == 0), stop=(hh == GHEADS - 1))
                            if hh == GHEADS - 1:
                                dst = ostage[:, oc * 512:(oc + 1) * 512]
                                if i % 2 == 0:
                                    nc.vector.tensor_copy(dst, ps[:])
                                else:
                                    nc.scalar.activation(dst, ps[:], AF.Copy)
                                nc.sync.dma_start(
                                    out[mq][:, oc * 512:(oc + 1) * 512], dst)
                        yield mm

            ostages = [outp.tile([128, HIDDEN], fp16, tag="out", name=f"os{i}")
                       for i in range(2)]

            bi = 0
            for h in range(GHEADS):
                if h < GHEADS - 1:
                    filler = proj_filler(h + 1)
                    per_stage = 4
                else:
                    wo_sb = wvo.tile([128, GHEADS * HIDDEN], fp16, tag="wvo",
                                     name="wo")
                    nc.sync.dma_start(wo_sb[:], woT)
                    filler = None  # switched per q-block below
                    per_stage = 8
                for qb in range(QB):
                    if h == GHEADS - 1 and qb >= 1:
                        filler = oproj_filler(qb - 1)
                    hq = h * SEQ
                    q0 = qb * 512
                    ep = epp.tile([128, KC // 2 * 512], fp16, tag="ep")
                    if h < GHEADS - 1:
                        sums = sump.tile([128, 512], fp32, tag="sums")
                    else:
                        sums = ps_sum.tile([128, 512], fp32, tag="ps_sum")
                    ctxp = ps_ctx.tile([128, 512], fp32, tag="ps_ctx")
                    state[bi] = (h, qb, eblk, ep, sums, ctxp)
                    for kp in range(KC // 2):
                        for i in (0, 1):
                            kc = 2 * kp + i
                            sc = ps_sc.tile([128, 512], fp32, tag="ps_sc")
                            nc.tensor.matmul(
                                sc[:],
                                kT_sb[:, hq + kc * 128: hq + (kc + 1) * 128],
                                qT_sb[:, hq + q0: hq + q0 + 512],
                                start=True, stop=True)
                            nc.scalar.activation(
                                eblk[:, kc * 512:(kc + 1) * 512], sc[:], AF.Exp)
                        nc.vector.tensor_add(
                            ep[:, kp * 512:(kp + 1) * 512],
                            eblk[:, (2 * kp) * 512:(2 * kp + 1) * 512],
                            eblk[:, (2 * kp + 1) * 512:(2 * kp + 2) * 512])
                        if kp % 2 == 1:
                            nc.vector.tensor_add(
                                ep[:, (kp - 1) * 512: kp * 512],
                                ep[:, (kp - 1) * 512: kp * 512],
                                ep[:, kp * 512:(kp + 1) * 512])
                        if kp % 4 == 3:
                            nc.vector.tensor_add(
                                ep[:, (kp - 3) * 512:(kp - 2) * 512],
                                ep[:, (kp - 3) * 512:(kp - 2) * 512],
                                ep[:, (kp - 1) * 512: kp * 512])
                        if kp == KC // 2 - 1:
                            nc.vector.tensor_add(
                                ep[:, 0:512], ep[:, 0:512],
                                ep[:, 4 * 512:5 * 512])
                        for b_kp in pend:
                            drain(*b_kp)
                        pend = [(bi, kp)]
                        if filler is not None:
                            for _ in range(per_stage):
                                mm = next(filler, None)
                                if mm is not None:
                                    mm()
                    if h == GHEADS - 1 and filler is not None:
                        for mm in filler:  # defensive: never drop filler work
                            mm()
                    bi += 1
                if h < GHEADS - 1 and filler is not None:
                    for mm in filler:
                        mm()
            for b_kp in pend:
                drain(*b_kp)

            # ---------- P4: leftover o_proj (last q-block) ----------
            # interleave tiles in groups of 3 (one per PSUM pool), with each
            # tile's hh=3 matmul deferred so the last ctx normalize (recip +
            # mul on VectorE) is off the PE critical path
            p4 = list(oproj_filler(QB - 1, deep=True))
            order = []
            for g in range(0, 16, 3):
                tiles = [p4[t * 4:(t + 1) * 4] for t in range(g, min(g + 3, 16))]
                for tl in tiles:
                    order += tl[:3]
                for tl in tiles:
                    order.append(tl[3])
            for mm in order:
                mm()

    nc.compile()
    return nc


def kernel(x, wq, bq, wk, bk, wv, bv, wo, bo):
    from concourse import bass_utils

    if "nc" not in _CACHE:
        _CACHE["nc"] = _build()
    nc = _CACHE["nc"]

    x = np.asarray(x, np.float32)
    wq = np.asarray(wq, np.float32)
    wk = np.asarray(wk, np.float32)
    wv = np.asarray(wv, np.float32)
    wo = np.asarray(wo, np.float32)
    scale = np.float32(1.0 / np.sqrt(HEAD_DIM))

    # xt[sb, t, p, c*128+s'] = x[b, sb*512+t*128+s', c*128+p]
    xT = [np.ascontiguousarray(
        x[b].reshape(SB, 4, 128, KC, 128).transpose(0, 1, 4, 3, 2)
        .reshape(SB, 4, 128, KC * 128)).astype(np.float16) for b in range(BATCH)]

    in_maps = []
    for j in range(N_CORES):
        b, g = divmod(j, GROUPS)
        ds = slice(g * GDIM, (g + 1) * GDIM)
        wq_s = (wq[ds] * scale).reshape(GHEADS, 128, KC, 128).transpose(0, 3, 2, 1)
        wk_s = wk[ds].reshape(GHEADS, 128, KC, 128).transpose(0, 3, 2, 1)
        wv_s = wv[ds].reshape(GDIM, KC, 128).transpose(2, 1, 0)
        wo_s = wo[:, ds].T.reshape(GHEADS, 128, HIDDEN).transpose(1, 0, 2)
        in_maps.append({
            "xt": xT[b],
            "wqt": np.ascontiguousarray(
                wq_s.reshape(GHEADS, 128, KC * 128)).astype(np.float16),
            "wkt": np.ascontiguousarray(
                wk_s.reshape(GHEADS, 128, KC * 128)).astype(np.float16),
            "wvt": np.ascontiguousarray(
                wv_s.reshape(128, KC * GDIM)).astype(np.float16),
            "wot": np.ascontiguousarray(
                wo_s.reshape(128, GHEADS * HIDDEN)).astype(np.float16),
            "bq": (np.asarray(bq)[ds] * scale).astype(np.float32),
        })

    res = bass_utils.run_bass_kernel_spmd(
        nc, in_maps, core_ids=list(range(N_CORES)),
        **_CACHE.get("run_kwargs", {}))
    _CACHE["last_res"] = res

    outp = np.zeros((BATCH, MT, 128, HIDDEN), np.float32)
    for j in range(N_CORES):
        b = j // GROUPS
        outp[b] += res.results[j]["out"].astype(np.float32)
    outp = outp.reshape(BATCH, SEQ, HIDDEN)
    bo_eff = np.asarray(bo, np.float32) + wo @ np.asarray(bv, np.float32)
    return outp + bo_eff


# revision 14
# speedup vs baseline: 1.0074x; 1.0074x over previous
"""Multi-head attention (B=2, S=2048, H=2048, 16 heads, d=128) on 8 TRN2
NeuronCores.

Sharding: 2-way batch x 4-way head-group tensor parallel. Core j handles
batch j//4 and heads 4*(j%4)..4*(j%4)+3 (a 512-wide slice of the qkv
projection output dim / o_proj input dim). Each core returns a partial
o_proj output [S, H] in fp16; the host sums the 4 partials per batch and
adds an effective bias bo + wo@bv (softmax rows sum to 1, so the v bias
contributes a constant; bk cancels inside softmax and is dropped).

All fp16 matmul operands, fp32 PSUM accumulation. DRAM layouts keep
16KB-contiguous per-partition rows (~420GB/s DMA vs ~200 for 1KB rows);
x block 0 is split into 4 seq-quarter DMAs so the first v tile starts
~5us earlier. One interleaved PE stream:

  P1: v = x@wv.T and head 0's qT/kT projections, seq-blocked, while x
      streams in.
  P2: for h in 0..2: attention(h) stages with head h+1's qT/kT projection
      matmuls as per-stage PE filler (x re-streamed from HBM per head).
  P3: attention(h=3) with o_proj matmuls of completed q-blocks as filler.
  P4: leftover o_proj through all idle PSUM banks.

Attention per (head, q-block), scoresT layout [k, q]:
  scoresT = kT_h.T @ qT_h -> exp on ScalarE -> expT (fp16)
  VectorE pair-adds exp chunks; sums += ones128.T @ pair  (half-cost rowsum)
  ctxT += v_chunk.T @ expT ; ctxT_norm = ctxT * approx_recip(sums)
"""
import sys

if "/opt/trn_rl_repo" not in sys.path:
    sys.path.insert(0, "/opt/trn_rl_repo")

import numpy as np

HIDDEN = 2048
HEADS = 16
HEAD_DIM = 128
BATCH = 2
SEQ = 2048

N_CORES = 8
GROUPS = 4               # head groups (cores per batch)
GDIM = HIDDEN // GROUPS  # 512 dims per core
GHEADS = GDIM // HEAD_DIM  # 4 heads per core
KC = HIDDEN // 128       # 16 contraction chunks
SB = 4                   # seq blocks of 512
QB = SEQ // 512          # 4 q-blocks in attention
MT = SEQ // 128          # 16 seq tiles of 128

_CACHE = {}


def _build():
    import concourse.bacc as bacc
    import concourse.bass as bass
    import concourse.bass_isa as bass_isa
    import concourse.mybir as mybir
    import concourse.tile as tile

    fp16 = mybir.dt.float16
    fp32 = mybir.dt.float32
    AF = mybir.ActivationFunctionType

    nc = bacc.Bacc("TRN2", target_bir_lowering=False, debug=False,
                   num_devices=N_CORES)

    # xt[sb, t, p, c*128+s'] = x[b, sb*512+t*128+s', c*128+p]
    xT = nc.dram_tensor("xt", [SB, 4, 128, KC * 128], fp16, kind="ExternalInput").ap()
    # wqt[h, p, c*128+m] = wq_scaled[h*128+m, c*128+p]
    wqT = nc.dram_tensor("wqt", [GHEADS, 128, KC * 128], fp16, kind="ExternalInput").ap()
    wkT = nc.dram_tensor("wkt", [GHEADS, 128, KC * 128], fp16, kind="ExternalInput").ap()
    # wvt[p, c*512+d] = wv[d, c*128+p]
    wvT = nc.dram_tensor("wvt", [128, KC * 512], fp16, kind="ExternalInput").ap()
    # wot[p, hh*2048+oc] = wo[oc, hh*128+p]
    woT = nc.dram_tensor("wot", [128, GHEADS * HIDDEN], fp16, kind="ExternalInput").ap()
    bq = nc.dram_tensor("bq", [GDIM], fp32, kind="ExternalInput").ap()
    # out[mq, p, oc] = partial[mq*128+p, oc], fp16
    out = nc.dram_tensor("out", [MT, 128, HIDDEN], fp16, kind="ExternalOutput").ap()

    with tile.TileContext(nc) as tc:
        with (
            tc.tile_pool(name="xp", bufs=2) as xp,        # 16KB x-block slots
            tc.tile_pool(name="wqk", bufs=2) as wqk,
            tc.tile_pool(name="wvo", bufs=2) as wvo,
            tc.tile_pool(name="res", bufs=1) as res,
            tc.tile_pool(name="ebp", bufs=1) as ebp,
            tc.tile_pool(name="epp", bufs=2) as epp,
            tc.tile_pool(name="small", bufs=1) as small,
            tc.tile_pool(name="rec", bufs=2) as rec,
            tc.tile_pool(name="sump", bufs=1) as sump,
            tc.tile_pool(name="outp", bufs=2) as outp,
            tc.tile_pool(name="ps_a", bufs=2, space=bass.MemorySpace.PSUM) as ps_a,
            tc.tile_pool(name="ps_sc", bufs=2, space=bass.MemorySpace.PSUM) as ps_sc,
            tc.tile_pool(name="ps_sum", bufs=2, space=bass.MemorySpace.PSUM) as ps_sum,
            tc.tile_pool(name="ps_ctx", bufs=2, space=bass.MemorySpace.PSUM) as ps_ctx,
        ):
            wq_sb = wqk.tile([128, GHEADS * KC * 128], fp16, tag="wqk", name="wq")
            wk_sb = wqk.tile([128, GHEADS * KC * 128], fp16, tag="wqk", name="wk")
            wv_sb = wvo.tile([128, KC * GDIM], fp16, tag="wvo", name="wv")

            qT_sb = res.tile([128, GHEADS * SEQ], fp16, tag="qT")
            kT_sb = res.tile([128, GHEADS * SEQ], fp16, tag="kT")
            v_sb = res.tile([128, MT * GDIM], fp16, tag="v")
            ctx_sb = res.tile([128, GHEADS * SEQ], fp16, tag="ctx")

            eblk = ebp.tile([128, KC * 512], fp16, tag="eblk")

            bq_sb = small.tile([128, GHEADS], fp32, tag="bq")
            ones_sb = small.tile([128, 128], fp16, tag="ones")
            nc.vector.memset(ones_sb[:], 1.0)

            # ---------- P0: initial DMAs + HAM warmup ----------
            # wv + x0 first (v tiles gate P1); x0 split by seq-quarter so
            # v_tile(0,0) starts after ~1.5MB instead of 3MB.
            xv = {0: xp.tile([128, 4 * KC * 128], fp16, tag="xp", name="x0")}
            nc.sync.dma_start(wv_sb[:, 0:4 * GDIM], wvT[:, 0:4 * GDIM])
            for t in range(4):
                nc.sync.dma_start(
                    xv[0][:, t * KC * 128:(t + 1) * KC * 128], xT[0, t])
                if t < 3:
                    nc.sync.dma_start(
                        wv_sb[:, (t + 1) * 4 * GDIM:(t + 2) * 4 * GDIM],
                        wvT[:, (t + 1) * 4 * GDIM:(t + 2) * 4 * GDIM])
            nc.sync.dma_start(bq_sb[:], bq.rearrange("(m p) -> p m", p=128))
            nc.sync.dma_start(wq_sb[:, 0:KC * 128], wqT[0])
            nc.sync.dma_start(wk_sb[:, 0:KC * 128], wkT[0])

            warm = ps_a.tile([128, 512], fp32, tag="ps_a", name="warm")
            for _ in range(48):
                nc.tensor.matmul(warm[:, :128], ones_sb[:], ones_sb[:],
                                 start=True, stop=True)

            def warm_trickle(n):
                wt = ps_ctx.tile([128, 64], fp32, tag="ps_ctx", name="wt")
                for _ in range(n):
                    nc.tensor.matmul(wt[:], ones_sb[:], ones_sb[:, :64],
                                     start=True, stop=True)

            # ---------- helpers ----------
            TQ = KC * 128  # 2048 cols per seq-quarter in an x block

            def v_tile(xblk, st):
                """One [128 seq, 512 dims] v tile (seq tile st = sb*4+t)."""
                t = st % 4
                ps = ps_a.tile([128, 512], fp32, tag="ps_a")
                for c in range(KC):
                    nc.tensor.matmul(
                        ps[:],
                        xblk[:, t * TQ + c * 128: t * TQ + (c + 1) * 128],
                        wv_sb[:, c * GDIM:(c + 1) * GDIM],
                        start=(c == 0), stop=(c == KC - 1))
                nc.vector.tensor_copy(v_sb[:, st * GDIM:(st + 1) * GDIM], ps[:])

            def xmov(xblk, c):
                """Moving-operand AP for chunk c over a 512-seq block:
                [128, 4 quarters, 128] strided view."""
                return xblk[:].rearrange(
                    "p (t r) -> p t r", t=4)[:, :, c * 128:(c + 1) * 128]

            def qk_tile(xblk, w_sb, b_sb, dst, h, s0, nm):
                """One [128 dims, 512 seq] q/k projection tile + bias copy."""
                ps = ps_sc.tile([128, 512], fp32, tag="ps_sc", name=f"pp{nm}")
                hw = h * KC * 128
                for c in range(KC):
                    nc.tensor.matmul(
                        ps[:],
                        w_sb[:, hw + c * 128: hw + (c + 1) * 128],
                        xmov(xblk, c),
                        start=(c == 0), stop=(c == KC - 1))
                nc.scalar.activation(
                    dst[:, h * SEQ + s0: h * SEQ + s0 + 512],
                    ps[:], AF.Identity,
                    bias=(b_sb[:, h:h + 1] if b_sb is not None else 0.0))

            # ---------- P1: v projection + head-0 q/k projection ----------
            for sb in range(SB):
                for t in range(4):
                    v_tile(xv[sb], sb * 4 + t)
                    if sb == 0 and t < 3:
                        warm_trickle(6)
                    if t == 1 and sb + 1 < SB:
                        xv[sb + 1] = xp.tile([128, 4 * TQ], fp16, tag="xp",
                                             name=f"x{sb + 1}")
                        nc.sync.dma_start(
                            xv[sb + 1][:].rearrange("p (t r) -> p t r", t=4),
                            xT[sb + 1].rearrange("t p r -> p t r"))
                    if t == 3 and sb == 0:
                        for h in range(1, GHEADS):
                            nc.sync.dma_start(
                                wq_sb[:, h * KC * 128:(h + 1) * KC * 128],
                                wqT[h])
                            nc.sync.dma_start(
                                wk_sb[:, h * KC * 128:(h + 1) * KC * 128],
                                wkT[h])
                qk_tile(xv[sb], wq_sb, bq_sb, qT_sb, 0, sb * 512, f"q{sb}")
                qk_tile(xv[sb], wk_sb, None, kT_sb, 0, sb * 512, f"k{sb}")

            # ---------- P2/P3: attention windows with PE filler ----------
            state = {}
            pend = []

            def drain(bi, kp):
                h, qb, eblk_, ep, sums, ctxp = state[bi]
                for kc in (2 * kp, 2 * kp + 1):
                    nc.tensor.matmul(ctxp[:],
                                     v_sb[:, kc * GDIM + h * 128:
                                          kc * GDIM + (h + 1) * 128],
                                     eblk_[:, kc * 512:(kc + 1) * 512],
                                     start=(kc == 0), stop=(kc == KC - 1))
                if kp == KC // 2 - 1:
                    if h < GHEADS - 1:
                        # off the critical path: idle GpSimd does the rowsum
                        nc.gpsimd.partition_all_reduce(
                            sums[:], ep[:, 0:512], 128, bass_isa.ReduceOp.add)
                    else:
                        # o_proj filler needs ctx promptly: fast PE rowsum
                        nc.tensor.matmul(sums[:], ones_sb[:],
                                         ep[:, 0:512], start=True, stop=True)
                    finish(bi)

            def finish(bi):
                h, qb, eblk_, ep, sums, ctxp = state.pop(bi)
                q0 = qb * 512
                recip = rec.tile([128, 512], fp32, tag="recip")
                nc.vector.reciprocal_approx_fast(recip[:], sums[:])
                nc.vector.tensor_mul(ctx_sb[:, h * SEQ + q0: h * SEQ + q0 + 512],
                                     ctxp[:], recip[:])

            # filler generators -------------------------------------------
            def proj_filler(h):
                """Yield 128 single-MM closures projecting head h's qT/kT,
                with x re-streamed per seq block (xp slots cycle)."""
                xb = {}

                def load_x(sb):
                    t = xp.tile([128, 4 * TQ], fp16, tag="xp",
                                name=f"xh{h}_{sb}")
                    nc.sync.dma_start(
                        t[:].rearrange("p (t r) -> p t r", t=4),
                        xT[sb].rearrange("t p r -> p t r"))
                    return t

                xb[0] = load_x(0)
                hw = h * KC * 128
                for sb in range(SB):
                    if sb + 1 < SB:
                        xb[sb + 1] = load_x(sb + 1)
                    s0 = sb * 512
                    for w_sb, b_sb, dst, nm in ((wq_sb, bq_sb, qT_sb, "q"),
                                                (wk_sb, None, kT_sb, "k")):
                        ps = ps_a.tile([128, 512], fp32, tag="ps_a",
                                       name=f"p{nm}{h}_{sb}")
                        for c in range(KC):
                            def mm(c=c, ps=ps, w_sb=w_sb, b_sb=b_sb, dst=dst,
                                   sb=sb, s0=s0):
                                nc.tensor.matmul(
                                    ps[:],
                                    w_sb[:, hw + c * 128: hw + (c + 1) * 128],
                                    xmov(xb[sb], c),
                                    start=(c == 0), stop=(c == KC - 1))
                                if c == KC - 1:
                                    nc.scalar.activation(
                                        dst[:, h * SEQ + s0: h * SEQ + s0 + 512],
                                        ps[:], AF.Identity,
                                        bias=(b_sb[:, h:h + 1]
                                              if b_sb is not None else 0.0))
                            yield mm

            def oproj_filler(qb, deep=False):
                """Yield 64 single-MM closures for o_proj q-tiles of block qb
                (all heads' ctx for qb must be finished)."""
                for i, (mq, oc) in enumerate(
                        (mq, oc) for mq in range(qb * 4, qb * 4 + 4)
                        for oc in range(4)):
                    if deep and i % 3 == 1:
                        ps = ps_sum.tile([128, 512], fp32, tag="ps_sum",
                                         name=f"po{mq}_{oc}")
                    elif deep and i % 3 == 2:
                        ps = ps_ctx.tile([128, 512], fp32, tag="ps_ctx",
                                         name=f"po{mq}_{oc}")
                    else:
                        ps = ps_a.tile([128, 512], fp32, tag="ps_a",
                                       name=f"po{mq}_{oc}")
                    ostage = ostages[mq % 2]
                    for hh in range(GHEADS):
                        def mm(ps=ps, hh=hh, mq=mq, oc=oc, i=i, ostage=ostage):
                            nc.tensor.matmul(
                                ps[:],
                                ctx_sb[:, hh * SEQ + mq * 128:
                                       hh * SEQ + (mq + 1) * 128],
                                wo_sb[:, hh * HIDDEN + oc * 512:
                                      hh * HIDDEN + (oc + 1) * 512],
                                start=(hh == 0), stop=(hh == GHEADS - 1))
                            if hh == GHEADS - 1:
                                dst = ostage[:, oc * 512:(oc + 1) * 512]
                                if i % 2 == 0:
                                    nc.vector.tensor_copy(dst, ps[:])
                                else:
                                    nc.scalar.activation(dst, ps[:], AF.Copy)
                                nc.sync.dma_start(
                                    out[mq][:, oc * 512:(oc + 1) * 512], dst)
                        yield mm

            ostages = [outp.tile([128, HIDDEN], fp16, tag="out", name=f"os{i}")
                       for i in range(2)]

            bi = 0
            for h in range(GHEADS):
                if h < GHEADS - 1:
                    filler = proj_filler(h + 1)
                    per_stage = 4
                else:
                    wo_sb = wvo.tile([128, GHEADS * HIDDEN], fp16, tag="wvo",
                                     name="wo")
                    nc.sync.dma_start(wo_sb[:], woT)
                    filler = None  # switched per q-block below
                    per_stage = 8
                for qb in range(QB):
                    if h == GHEADS - 1 and qb >= 1:
                        filler = oproj_filler(qb - 1)
                    hq = h * SEQ
                    q0 = qb * 512
                    ep = epp.tile([128, KC // 2 * 512], fp16, tag="ep")
                    if h < GHEADS - 1:
                        sums = sump.tile([128, 512], fp32, tag="sums")
                    else:
                        sums = ps_sum.tile([128, 512], fp32, tag="ps_sum")
                    ctxp = ps_ctx.tile([128, 512], fp32, tag="ps_ctx")
                    state[bi] = (h, qb, eblk, ep, sums, ctxp)
                    for kp in range(KC // 2):
                        for i in (0, 1):
                            kc = 2 * kp + i
                            sc = ps_sc.tile([128, 512], fp32, tag="ps_sc")
                            nc.tensor.matmul(
                                sc[:],
                                kT_sb[:, hq + kc * 128: hq + (kc + 1) * 128],
                                qT_sb[:, hq + q0: hq + q0 + 512],
                                start=True, stop=True)
                            nc.scalar.activation(
                                eblk[:, kc * 512:(kc + 1) * 512], sc[:], AF.Exp)
                        nc.vector.tensor_add(
                            ep[:, kp * 512:(kp + 1) * 512],
                            eblk[:, (2 * kp) * 512:(2 * kp + 1) * 512],
                            eblk[:, (2 * kp + 1) * 512:(2 * kp + 2) * 512])
                        if kp % 2 == 1:
                            nc.vector.tensor_add(
                                ep[:, (kp - 1) * 512: kp * 512],
                                ep[:, (kp - 1) * 512: kp * 512],
                                ep[:, kp * 512:(kp + 1) * 512])
                        if kp % 4 == 3:
                            nc.vector.tensor_add(
                                ep[:, (kp - 3) * 512:(kp - 2) * 512],
                                ep[:, (kp - 3) * 512:(kp - 2) * 512],
                                ep[:, (kp - 1) * 512: kp * 512])
                        if kp == KC // 2 - 1:
                            nc.vector.tensor_add(
                                ep[:, 0:512], ep[:, 0:512],
                                ep[:, 4 * 512:5 * 512])
                        for b_kp in pend:
                            drain(*b_kp)
                        pend = [(bi, kp)]
                        if filler is not None:
                            for _ in range(per_stage):
                                mm = next(filler, None)
                                if mm is not None:
                                    mm()
                    if h == GHEADS - 1 and filler is not None:
                        for mm in filler:  # defensive: never drop filler work
                            mm()
                    bi += 1
                if h < GHEADS - 1 and filler is not None:
                    for mm in filler:
                        mm()
            for b_kp in pend:
                drain(*b_kp)

            # ---------- P4: leftover o_proj (last q-block) ----------
            # interleave tiles in groups of 3 (one per PSUM pool), with each
            # tile's hh=3 matmul deferred so the last ctx normalize (recip +
            # mul on VectorE) is off the PE critical path
            p4 = list(oproj_filler(QB - 1, deep=True))
            order = []
            for g in range(0, 16, 3):
                tiles = [p4[t * 4:(t + 1) * 4] for t in range(g, min(g + 3, 16))]
                for tl in tiles:
                    order += tl[:3]
                for tl in tiles:
                    order.append(tl[3])
            for mm in order:
                mm()

    nc.compile()
    return nc


def kernel(x, wq, bq, wk, bk, wv, bv, wo, bo):
    from concourse import bass_utils

    if "nc" not in _CACHE:
        _CACHE["nc"] = _build()
    nc = _CACHE["nc"]

    x = np.asarray(x, np.float32)
    wq = np.asarray(wq, np.float32)
    wk = np.asarray(wk, np.float32)
    wv = np.asarray(wv, np.float32)
    wo = np.asarray(wo, np.float32)
    scale = np.float32(1.0 / np.sqrt(HEAD_DIM))

    # xt[sb, t, p, c*128+s'] = x[b, sb*512+t*128+s', c*128+p]
    xT = [np.ascontiguousarray(
        x[b].reshape(SB, 4, 128, KC, 128).transpose(0, 1, 4, 3, 2)
        .reshape(SB, 4, 128, KC * 128)).astype(np.float16) for b in range(BATCH)]

    in_maps = []
    for j in range(N_CORES):
        b, g = divmod(j, GROUPS)
        ds = slice(g * GDIM, (g + 1) * GDIM)
        wq_s = (wq[ds] * scale).reshape(GHEADS, 128, KC, 128).transpose(0, 3, 2, 1)
        wk_s = wk[ds].reshape(GHEADS, 128, KC, 128).transpose(0, 3, 2, 1)
        wv_s = wv[ds].reshape(GDIM, KC, 128).transpose(2, 1, 0)
        wo_s = wo[:, ds].T.reshape(GHEADS, 128, HIDDEN).transpose(1, 0, 2)
        in_maps.append({
            "xt": xT[b],
            "wqt": np.ascontiguousarray(
                wq_s.reshape(GHEADS, 128, KC * 128)).astype(np.float16),
            "wkt": np.ascontiguousarray(
                wk_s.reshape(GHEADS, 128, KC * 128)).astype(np.float16),
            "wvt": np.ascontiguousarray(
                wv_s.reshape(128, KC * GDIM)).astype(np.float16),
            "wot": np.ascontiguousarray(
                wo_s.reshape(128, GHEADS * HIDDEN)).astype(np.float16),
            "bq": (np.asarray(bq)[ds] * scale).astype(np.float32),
        })

    res = bass_utils.run_bass_kernel_spmd(
        nc, in_maps, core_ids=list(range(N_CORES)),
        **_CACHE.get("run_kwargs", {}))
    _CACHE["last_res"] = res

    outp = np.zeros((BATCH, MT, 128, HIDDEN), np.float32)
    for j in range(N_CORES):
        b = j // GROUPS
        outp[b] += res.results[j]["out"].astype(np.float32)
    outp = outp.reshape(BATCH, SEQ, HIDDEN)
    bo_eff = np.asarray(bo, np.float32) + wo @ np.asarray(bv, np.float32)
    return outp + bo_eff


# revision 15
# speedup vs baseline: 1.0094x; 1.0020x over previous
"""Multi-head attention (B=2, S=2048, H=2048, 16 heads, d=128) on 8 TRN2
NeuronCores.

Sharding: 2-way batch x 4-way head-group tensor parallel. Core j handles
batch j//4 and heads 4*(j%4)..4*(j%4)+3 (a 512-wide slice of the qkv
projection output dim / o_proj input dim). Each core returns a partial
o_proj output [S, H] in fp16; the host sums the 4 partials per batch and
adds an effective bias bo + wo@bv (softmax rows sum to 1, so the v bias
contributes a constant; bk cancels inside softmax and is dropped).

All fp16 matmul operands, fp32 PSUM accumulation. DRAM layouts keep
16KB-contiguous per-partition rows (~420GB/s DMA vs ~200 for 1KB rows);
x block 0 is split into 4 seq-quarter DMAs so the first v tile starts
~5us earlier. One interleaved PE stream:

  P1: v = x@wv.T and head 0's qT/kT projections, seq-blocked, while x
      streams in.
  P2: for h in 0..2: attention(h) stages with head h+1's qT/kT projection
      matmuls as per-stage PE filler (x re-streamed from HBM per head).
  P3: attention(h=3) with o_proj matmuls of completed q-blocks as filler.
  P4: leftover o_proj through all idle PSUM banks.

Attention per (head, q-block), scoresT layout [k, q]:
  scoresT = kT_h.T @ qT_h -> exp on ScalarE -> expT (fp16)
  VectorE pair-adds exp chunks; sums += ones128.T @ pair  (half-cost rowsum)
  ctxT += v_chunk.T @ expT ; ctxT_norm = ctxT * approx_recip(sums)
"""
import sys

if "/opt/trn_rl_repo" not in sys.path:
    sys.path.insert(0, "/opt/trn_rl_repo")

import numpy as np

HIDDEN = 2048
HEADS = 16
HEAD_DIM = 128
BATCH = 2
SEQ = 2048

N_CORES = 8
GROUPS = 4               # head groups (cores per batch)
GDIM = HIDDEN // GROUPS  # 512 dims per core
GHEADS = GDIM // HEAD_DIM  # 4 heads per core
KC = HIDDEN // 128       # 16 contraction chunks
SB = 4                   # seq blocks of 512
QB = SEQ // 512          # 4 q-blocks in attention
MT = SEQ // 128          # 16 seq tiles of 128

_CACHE = {}


def _build():
    import concourse.bacc as bacc
    import concourse.bass as bass
    import concourse.bass_isa as bass_isa
    import concourse.mybir as mybir
    import concourse.tile as tile

    fp16 = mybir.dt.float16
    fp32 = mybir.dt.float32
    AF = mybir.ActivationFunctionType

    nc = bacc.Bacc("TRN2", target_bir_lowering=False, debug=False,
                   num_devices=N_CORES)

    # xt[sb, t, p, c*128+s'] = x[b, sb*512+t*128+s', c*128+p]
    xT = nc.dram_tensor("xt", [SB, 4, 128, KC * 128], fp16, kind="ExternalInput").ap()
    # wqt[h, p, c*128+m] = wq_scaled[h*128+m, c*128+p]
    wqT = nc.dram_tensor("wqt", [GHEADS, 128, KC * 128], fp16, kind="ExternalInput").ap()
    wkT = nc.dram_tensor("wkt", [GHEADS, 128, KC * 128], fp16, kind="ExternalInput").ap()
    # wvt[p, c*512+d] = wv[d, c*128+p]
    wvT = nc.dram_tensor("wvt", [128, KC * 512], fp16, kind="ExternalInput").ap()
    # wot[p, hh*2048+oc] = wo[oc, hh*128+p]
    woT = nc.dram_tensor("wot", [128, GHEADS * HIDDEN], fp16, kind="ExternalInput").ap()
    bq = nc.dram_tensor("bq", [GDIM], fp32, kind="ExternalInput").ap()
    # out[mq, p, oc] = partial[mq*128+p, oc], fp16
    out = nc.dram_tensor("out", [MT, 128, HIDDEN], fp16, kind="ExternalOutput").ap()

    with tile.TileContext(nc) as tc:
        with (
            tc.tile_pool(name="xp", bufs=2) as xp,        # 16KB x-block slots
            tc.tile_pool(name="wqk", bufs=2) as wqk,
            tc.tile_pool(name="wvo", bufs=2) as wvo,
            tc.tile_pool(name="res", bufs=1) as res,
            tc.tile_pool(name="ebp", bufs=1) as ebp,
            tc.tile_pool(name="epp", bufs=2) as epp,
            tc.tile_pool(name="small", bufs=1) as small,
            tc.tile_pool(name="rec", bufs=2) as rec,
            tc.tile_pool(name="sump", bufs=1) as sump,
            tc.tile_pool(name="outp", bufs=2) as outp,
            tc.tile_pool(name="ps_a", bufs=2, space=bass.MemorySpace.PSUM) as ps_a,
            tc.tile_pool(name="ps_sc", bufs=2, space=bass.MemorySpace.PSUM) as ps_sc,
            tc.tile_pool(name="ps_sum", bufs=2, space=bass.MemorySpace.PSUM) as ps_sum,
            tc.tile_pool(name="ps_ctx", bufs=2, space=bass.MemorySpace.PSUM) as ps_ctx,
        ):
            wq_sb = wqk.tile([128, GHEADS * KC * 128], fp16, tag="wqk", name="wq")
            wk_sb = wqk.tile([128, GHEADS * KC * 128], fp16, tag="wqk", name="wk")
            wv_sb = wvo.tile([128, KC * GDIM], fp16, tag="wvo", name="wv")

            qT_sb = res.tile([128, GHEADS * SEQ], fp16, tag="qT")
            kT_sb = res.tile([128, GHEADS * SEQ], fp16, tag="kT")
            v_sb = res.tile([128, MT * GDIM], fp16, tag="v")
            ctx_sb = res.tile([128, GHEADS * SEQ], fp16, tag="ctx")

            eblk = ebp.tile([128, KC * 512], fp16, tag="eblk")

            bq_sb = small.tile([128, GHEADS], fp32, tag="bq")
            ones_sb = small.tile([128, 128], fp16, tag="ones")
            nc.vector.memset(ones_sb[:], 1.0)

            # ---------- P0: initial DMAs + HAM warmup ----------
            # wv + x0 first (v tiles gate P1); x0 split by seq-quarter so
            # v_tile(0,0) starts after ~1.5MB instead of 3MB.
            xv = {0: xp.tile([128, 4 * KC * 128], fp16, tag="xp", name="x0")}
            nc.sync.dma_start(wv_sb[:, 0:4 * GDIM], wvT[:, 0:4 * GDIM])
            for t in range(4):
                nc.sync.dma_start(
                    xv[0][:, t * KC * 128:(t + 1) * KC * 128], xT[0, t])
                if t < 3:
                    nc.sync.dma_start(
                        wv_sb[:, (t + 1) * 4 * GDIM:(t + 2) * 4 * GDIM],
                        wvT[:, (t + 1) * 4 * GDIM:(t + 2) * 4 * GDIM])
            nc.sync.dma_start(bq_sb[:], bq.rearrange("(m p) -> p m", p=128))
            nc.sync.dma_start(wq_sb[:, 0:KC * 128], wqT[0])
            nc.sync.dma_start(wk_sb[:, 0:KC * 128], wkT[0])

            warm = ps_a.tile([128, 512], fp32, tag="ps_a", name="warm")
            for _ in range(48):
                nc.tensor.matmul(warm[:, :128], ones_sb[:], ones_sb[:],
                                 start=True, stop=True)

            def warm_trickle(n):
                wt = ps_ctx.tile([128, 64], fp32, tag="ps_ctx", name="wt")
                for _ in range(n):
                    nc.tensor.matmul(wt[:], ones_sb[:], ones_sb[:, :64],
                                     start=True, stop=True)

            # ---------- helpers ----------
            TQ = KC * 128  # 2048 cols per seq-quarter in an x block

            def v_tile(xblk, st):
                """One [128 seq, 512 dims] v tile (seq tile st = sb*4+t)."""
                t = st % 4
                ps = ps_a.tile([128, 512], fp32, tag="ps_a")
                for c in range(KC):
                    nc.tensor.matmul(
                        ps[:],
                        xblk[:, t * TQ + c * 128: t * TQ + (c + 1) * 128],
                        wv_sb[:, c * GDIM:(c + 1) * GDIM],
                        start=(c == 0), stop=(c == KC - 1))
                nc.vector.tensor_copy(v_sb[:, st * GDIM:(st + 1) * GDIM], ps[:])

            def xmov(xblk, c):
                """Moving-operand AP for chunk c over a 512-seq block:
                [128, 4 quarters, 128] strided view."""
                return xblk[:].rearrange(
                    "p (t r) -> p t r", t=4)[:, :, c * 128:(c + 1) * 128]

            def qk_tile(xblk, w_sb, b_sb, dst, h, s0, nm):
                """One [128 dims, 512 seq] q/k projection tile + bias copy."""
                ps = ps_sc.tile([128, 512], fp32, tag="ps_sc", name=f"pp{nm}")
                hw = h * KC * 128
                for c in range(KC):
                    nc.tensor.matmul(
                        ps[:],
                        w_sb[:, hw + c * 128: hw + (c + 1) * 128],
                        xmov(xblk, c),
                        start=(c == 0), stop=(c == KC - 1))
                nc.scalar.activation(
                    dst[:, h * SEQ + s0: h * SEQ + s0 + 512],
                    ps[:], AF.Identity,
                    bias=(b_sb[:, h:h + 1] if b_sb is not None else 0.0))

            # ---------- P1: v projection + head-0 q/k projection ----------
            for sb in range(SB):
                for t in range(4):
                    v_tile(xv[sb], sb * 4 + t)
                    if sb == 0 and t < 3:
                        warm_trickle(6)
                    if t == 1 and sb + 1 < SB:
                        xv[sb + 1] = xp.tile([128, 4 * TQ], fp16, tag="xp",
                                             name=f"x{sb + 1}")
                        nc.sync.dma_start(
                            xv[sb + 1][:].rearrange("p (t r) -> p t r", t=4),
                            xT[sb + 1].rearrange("t p r -> p t r"))
                    if t == 3 and sb == 0:
                        for h in range(1, GHEADS):
                            nc.sync.dma_start(
                                wq_sb[:, h * KC * 128:(h + 1) * KC * 128],
                                wqT[h])
                            nc.sync.dma_start(
                                wk_sb[:, h * KC * 128:(h + 1) * KC * 128],
                                wkT[h])
                qk_tile(xv[sb], wq_sb, bq_sb, qT_sb, 0, sb * 512, f"q{sb}")
                qk_tile(xv[sb], wk_sb, None, kT_sb, 0, sb * 512, f"k{sb}")

            # ---------- P2/P3: attention windows with PE filler ----------
            state = {}
            pend = []

            def drain(bi, kp):
                h, qb, eblk_, ep, sums, ctxp = state[bi]
                for kc in (2 * kp, 2 * kp + 1):
                    nc.tensor.matmul(ctxp[:],
                                     v_sb[:, kc * GDIM + h * 128:
                                          kc * GDIM + (h + 1) * 128],
                                     eblk_[:, kc * 512:(kc + 1) * 512],
                                     start=(kc == 0), stop=(kc == KC - 1))
                if kp == KC // 2 - 1:
                    if h < GHEADS - 1:
                        # off the critical path: idle GpSimd does the rowsum
                        nc.gpsimd.partition_all_reduce(
                            sums[:], ep[:, 0:512], 128, bass_isa.ReduceOp.add)
                    else:
                        # o_proj filler needs ctx promptly: fast PE rowsum
                        nc.tensor.matmul(sums[:], ones_sb[:],
                                         ep[:, 0:512], start=True, stop=True)
                    finish(bi)

            def finish(bi):
                h, qb, eblk_, ep, sums, ctxp = state.pop(bi)
                q0 = qb * 512
                recip = rec.tile([128, 512], fp32, tag="recip")
                nc.vector.reciprocal_approx_fast(recip[:], sums[:])
                nc.vector.tensor_mul(ctx_sb[:, h * SEQ + q0: h * SEQ + q0 + 512],
                                     ctxp[:], recip[:])

            # filler generators -------------------------------------------
            def proj_filler(h):
                """Yield 128 single-MM closures projecting head h's qT/kT,
                with x re-streamed per seq block (xp slots cycle)."""
                xb = {}

                def load_x(sb):
                    t = xp.tile([128, 4 * TQ], fp16, tag="xp",
                                name=f"xh{h}_{sb}")
                    nc.sync.dma_start(
                        t[:].rearrange("p (t r) -> p t r", t=4),
                        xT[sb].rearrange("t p r -> p t r"))
                    return t

                xb[0] = load_x(0)
                hw = h * KC * 128
                for sb in range(SB):
                    if sb + 1 < SB:
                        xb[sb + 1] = load_x(sb + 1)
                    s0 = sb * 512
                    for w_sb, b_sb, dst, nm in ((wq_sb, bq_sb, qT_sb, "q"),
                                                (wk_sb, None, kT_sb, "k")):
                        ps = ps_a.tile([128, 512], fp32, tag="ps_a",
                                       name=f"p{nm}{h}_{sb}")
                        for c in range(KC):
                            def mm(c=c, ps=ps, w_sb=w_sb, b_sb=b_sb, dst=dst,
                                   sb=sb, s0=s0):
                                nc.tensor.matmul(
                                    ps[:],
                                    w_sb[:, hw + c * 128: hw + (c + 1) * 128],
                                    xmov(xb[sb], c),
                                    start=(c == 0), stop=(c == KC - 1))
                                if c == KC - 1:
                                    nc.scalar.activation(
                                        dst[:, h * SEQ + s0: h * SEQ + s0 + 512],
                                        ps[:], AF.Identity,
                                        bias=(b_sb[:, h:h + 1]
                                              if b_sb is not None else 0.0))
                            yield mm

            def oproj_filler(qb, deep=False, pre=None):
                """Yield single-MM closures for o_proj q-tiles of block qb
                (all heads' ctx for qb must be finished). Tiles in `pre`
                already hold hh=0..2 partials in a PSUM slot from the
                h3-qb0 prefill; only their hh=3 (+copy+DMA) is yielded,
                first, so their banks free before new allocations."""
                pre = pre or {}
                pairs = [(mq, oc) for mq in range(qb * 4, qb * 4 + 4)
                         for oc in range(4)]
                pairs.sort(key=lambda t: t not in pre)
                for i, (mq, oc) in enumerate(pairs):
                    if (mq, oc) in pre:
                        ps = pre[(mq, oc)]
                        ostage = ostages[mq % 2]

                        def mm3(ps=ps, mq=mq, oc=oc, i=i, ostage=ostage):
                            nc.tensor.matmul(
                                ps[:],
                                ctx_sb[:, 3 * SEQ + mq * 128:
                                       3 * SEQ + (mq + 1) * 128],
                                wo_sb[:, 3 * HIDDEN + oc * 512:
                                      3 * HIDDEN + (oc + 1) * 512],
                                start=False, stop=True)
                            dst = ostage[:, oc * 512:(oc + 1) * 512]
                            if i % 2 == 0:
                                nc.vector.tensor_copy(dst, ps[:])
                            else:
                                nc.scalar.activation(dst, ps[:], AF.Copy)
                            nc.sync.dma_start(
                                out[mq][:, oc * 512:(oc + 1) * 512], dst)
                        yield mm3
                        continue
                    if deep and i % 3 == 1:
                        ps = ps_sum.tile([128, 512], fp32, tag="ps_sum",
                                         name=f"po{mq}_{oc}")
                    elif deep and i % 3 == 2:
                        ps = ps_ctx.tile([128, 512], fp32, tag="ps_ctx",
                                         name=f"po{mq}_{oc}")
                    else:
                        ps = ps_a.tile([128, 512], fp32, tag="ps_a",
                                       name=f"po{mq}_{oc}")
                    ostage = ostages[mq % 2]
                    for hh in range(GHEADS):
                        def mm(ps=ps, hh=hh, mq=mq, oc=oc, i=i, ostage=ostage):
                            nc.tensor.matmul(
                                ps[:],
                                ctx_sb[:, hh * SEQ + mq * 128:
                                       hh * SEQ + (mq + 1) * 128],
                                wo_sb[:, hh * HIDDEN + oc * 512:
                                      hh * HIDDEN + (oc + 1) * 512],
                                start=(hh == 0), stop=(hh == GHEADS - 1))
                            if hh == GHEADS - 1:
                                dst = ostage[:, oc * 512:(oc + 1) * 512]
                                if i % 2 == 0:
                                    nc.vector.tensor_copy(dst, ps[:])
                                else:
                                    nc.scalar.activation(dst, ps[:], AF.Copy)
                                nc.sync.dma_start(
                                    out[mq][:, oc * 512:(oc + 1) * 512], dst)
                        yield mm

            ostages = [outp.tile([128, HIDDEN], fp16, tag="out", name=f"os{i}")
                       for i in range(2)]

            bi = 0
            for h in range(GHEADS):
                if h < GHEADS - 1:
                    filler = proj_filler(h + 1)
                    per_stage = 4
                else:
                    wo_sb = wvo.tile([128, GHEADS * HIDDEN], fp16, tag="wvo",
                                     name="wo")
                    nc.sync.dma_start(wo_sb[:], woT)
                    filler = None  # switched per q-block below
                    per_stage = 8
                    pre_state = {}
                    pre_mms = []
                    # h3-qb0 is ACT-bound (no filler available): prefill
                    # hh=0..2 o_proj partials for qb0 into idle PSUM slots
                    for (mq, oc, pool, ptag) in ((0, 0, ps_a, "ps_a"),
                                                 (0, 1, ps_a, "ps_a"),
                                                 (0, 2, ps_sum, "ps_sum")):
                        ps = pool.tile([128, 512], fp32, tag=ptag,
                                       name=f"pre{mq}_{oc}")
                        pre_state[(mq, oc)] = ps
                        for hh in range(3):
                            def mm(ps=ps, hh=hh, mq=mq, oc=oc):
                                nc.tensor.matmul(
                                    ps[:],
                                    ctx_sb[:, hh * SEQ + mq * 128:
                                           hh * SEQ + (mq + 1) * 128],
                                    wo_sb[:, hh * HIDDEN + oc * 512:
                                          hh * HIDDEN + (oc + 1) * 512],
                                    start=(hh == 0), stop=False)
                            pre_mms.append(mm)
                for qb in range(QB):
                    if h == GHEADS - 1 and qb >= 1:
                        filler = oproj_filler(qb - 1, pre=pre_state)
                        pre_state = {}
                    hq = h * SEQ
                    q0 = qb * 512
                    ep = epp.tile([128, KC // 2 * 512], fp16, tag="ep")
                    if h < GHEADS - 1:
                        sums = sump.tile([128, 512], fp32, tag="sums")
                    else:
                        sums = ps_sum.tile([128, 512], fp32, tag="ps_sum")
                    ctxp = ps_ctx.tile([128, 512], fp32, tag="ps_ctx")
                    state[bi] = (h, qb, eblk, ep, sums, ctxp)
                    for kp in range(KC // 2):
                        for i in (0, 1):
                            kc = 2 * kp + i
                            sc = ps_sc.tile([128, 512], fp32, tag="ps_sc")
                            nc.tensor.matmul(
                                sc[:],
                                kT_sb[:, hq + kc * 128: hq + (kc + 1) * 128],
                                qT_sb[:, hq + q0: hq + q0 + 512],
                                start=True, stop=True)
                            nc.scalar.activation(
                                eblk[:, kc * 512:(kc + 1) * 512], sc[:], AF.Exp)
                        nc.vector.tensor_add(
                            ep[:, kp * 512:(kp + 1) * 512],
                            eblk[:, (2 * kp) * 512:(2 * kp + 1) * 512],
                            eblk[:, (2 * kp + 1) * 512:(2 * kp + 2) * 512])
                        if kp % 2 == 1:
                            nc.vector.tensor_add(
                                ep[:, (kp - 1) * 512: kp * 512],
                                ep[:, (kp - 1) * 512: kp * 512],
                                ep[:, kp * 512:(kp + 1) * 512])
                        if kp % 4 == 3:
                            nc.vector.tensor_add(
                                ep[:, (kp - 3) * 512:(kp - 2) * 512],
                                ep[:, (kp - 3) * 512:(kp - 2) * 512],
                                ep[:, (kp - 1) * 512: kp * 512])
                        if kp == KC // 2 - 1:
                            nc.vector.tensor_add(
                                ep[:, 0:512], ep[:, 0:512],
                                ep[:, 4 * 512:5 * 512])
                        for b_kp in pend:
                            drain(*b_kp)
                        pend = [(bi, kp)]
                        if filler is not None:
                            for _ in range(per_stage):
                                mm = next(filler, None)
                                if mm is not None:
                                    mm()
                        elif h == GHEADS - 1 and pre_mms:
                            for mm in pre_mms[:2]:
                                mm()
                            del pre_mms[:2]
                    if h == GHEADS - 1 and filler is not None:
                        for mm in filler:  # defensive: never drop filler work
                            mm()
                    bi += 1
                if h < GHEADS - 1 and filler is not None:
                    for mm in filler:
                        mm()
            for b_kp in pend:
                drain(*b_kp)

            # ---------- P4: leftover o_proj (last q-block) ----------
            # interleave tiles in groups of 3 (one per PSUM pool), with each
            # tile's hh=3 matmul deferred so the last ctx normalize (recip +
            # mul on VectorE) is off the PE critical path
            p4 = list(oproj_filler(QB - 1, deep=True))
            order = []
            for g in range(0, 16, 3):
                tiles = [p4[t * 4:(t + 1) * 4] for t in range(g, min(g + 3, 16))]
                for tl in tiles:
                    order += tl[:3]
                for tl in tiles:
                    order.append(tl[3])
            for mm in order:
                mm()

    nc.compile()
    return nc


def kernel(x, wq, bq, wk, bk, wv, bv, wo, bo):
    from concourse import bass_utils

    if "nc" not in _CACHE:
        _CACHE["nc"] = _build()
    nc = _CACHE["nc"]

    x = np.asarray(x, np.float32)
    wq = np.asarray(wq, np.float32)
    wk = np.asarray(wk, np.float32)
    wv = np.asarray(wv, np.float32)
    wo = np.asarray(wo, np.float32)
    scale = np.float32(1.0 / np.sqrt(HEAD_DIM))

    # xt[sb, t, p, c*128+s'] = x[b, sb*512+t*128+s', c*128+p]
    xT = [np.ascontiguousarray(
        x[b].reshape(SB, 4, 128, KC, 128).transpose(0, 1, 4, 3, 2)
        .reshape(SB, 4, 128, KC * 128)).astype(np.float16) for b in range(BATCH)]

    in_maps = []
    for j in range(N_CORES):
        b, g = divmod(j, GROUPS)
        ds = slice(g * GDIM, (g + 1) * GDIM)
        wq_s = (wq[ds] * scale).reshape(GHEADS, 128, KC, 128).transpose(0, 3, 2, 1)
        wk_s = wk[ds].reshape(GHEADS, 128, KC, 128).transpose(0, 3, 2, 1)
        wv_s = wv[ds].reshape(GDIM, KC, 128).transpose(2, 1, 0)
        wo_s = wo[:, ds].T.reshape(GHEADS, 128, HIDDEN).transpose(1, 0, 2)
        in_maps.append({
            "xt": xT[b],
            "wqt": np.ascontiguousarray(
                wq_s.reshape(GHEADS, 128, KC * 128)).astype(np.float16),
            "wkt": np.ascontiguousarray(
                wk_s.reshape(GHEADS, 128, KC * 128)).astype(np.float16),
            "wvt": np.ascontiguousarray(
                wv_s.reshape(128, KC * GDIM)).astype(np.float16),
            "wot": np.ascontiguousarray(
                wo_s.reshape(128, GHEADS * HIDDEN)).astype(np.float16),
            "bq": (np.asarray(bq)[ds] * scale).astype(np.float32),
        })

    res = bass_utils.run_bass_kernel_spmd(
        nc, in_maps, core_ids=list(range(N_CORES)),
        **_CACHE.get("run_kwargs", {}))
    _CACHE["last_res"] = res

    outp = np.zeros((BATCH, MT, 128, HIDDEN), np.float32)
    for j in range(N_CORES):
        b = j // GROUPS
        outp[b] += res.results[j]["out"].astype(np.float32)
    outp = outp.reshape(BATCH, SEQ, HIDDEN)
    bo_eff = np.asarray(bo, np.float32) + wo @ np.asarray(bv, np.float32)
    return outp + bo_eff


# revision 16
# speedup vs baseline: 1.0120x; 1.0026x over previous
"""Multi-head attention (B=2, S=2048, H=2048, 16 heads, d=128) on 8 TRN2
NeuronCores.

Sharding: 2-way batch x 4-way head-group tensor parallel. Core j handles
batch j//4 and heads 4*(j%4)..4*(j%4)+3 (a 512-wide slice of the qkv
projection output dim / o_proj input dim). Each core returns a partial
o_proj output [S, H] in fp16; the host sums the 4 partials per batch and
adds an effective bias bo + wo@bv (softmax rows sum to 1, so the v bias
contributes a constant; bk cancels inside softmax and is dropped).

All fp16 matmul operands, fp32 PSUM accumulation. DRAM layouts keep
16KB-contiguous per-partition rows (~420GB/s DMA vs ~200 for 1KB rows);
x block 0 is split into 4 seq-quarter DMAs so the first v tile starts
~5us earlier. One interleaved PE stream:

  P1: v = x@wv.T and head 0's qT/kT projections, seq-blocked, while x
      streams in.
  P2: for h in 0..2: attention(h) stages with head h+1's qT/kT projection
      matmuls as per-stage PE filler (x re-streamed from HBM per head).
  P3: attention(h=3) with o_proj matmuls of completed q-blocks as filler.
  P4: leftover o_proj through all idle PSUM banks.

Attention per (head, q-block), scoresT layout [k, q]:
  scoresT = kT_h.T @ qT_h -> exp on ScalarE -> expT (fp16)
  VectorE pair-adds exp chunks; sums += ones128.T @ pair  (half-cost rowsum)
  ctxT += v_chunk.T @ expT ; ctxT_norm = ctxT * approx_recip(sums)
"""
import sys

if "/opt/trn_rl_repo" not in sys.path:
    sys.path.insert(0, "/opt/trn_rl_repo")

import numpy as np

HIDDEN = 2048
HEADS = 16
HEAD_DIM = 128
BATCH = 2
SEQ = 2048

N_CORES = 8
GROUPS = 4               # head groups (cores per batch)
GDIM = HIDDEN // GROUPS  # 512 dims per core
GHEADS = GDIM // HEAD_DIM  # 4 heads per core
KC = HIDDEN // 128       # 16 contraction chunks
SB = 4                   # seq blocks of 512
QB = SEQ // 512          # 4 q-blocks in attention
MT = SEQ // 128          # 16 seq tiles of 128

_CACHE = {}


def _build():
    import concourse.bacc as bacc
    import concourse.bass as bass
    import concourse.bass_isa as bass_isa
    import concourse.mybir as mybir
    import concourse.tile as tile

    fp16 = mybir.dt.float16
    fp32 = mybir.dt.float32
    AF = mybir.ActivationFunctionType

    nc = bacc.Bacc("TRN2", target_bir_lowering=False, debug=False,
                   num_devices=N_CORES)

    # xt[sb, t, p, c*128+s'] = x[b, sb*512+t*128+s', c*128+p]
    xT = nc.dram_tensor("xt", [SB, 4, 128, KC * 128], fp16, kind="ExternalInput").ap()
    # wqt[h, p, c*128+m] = wq_scaled[h*128+m, c*128+p]
    wqT = nc.dram_tensor("wqt", [GHEADS, 128, KC * 128], fp16, kind="ExternalInput").ap()
    wkT = nc.dram_tensor("wkt", [GHEADS, 128, KC * 128], fp16, kind="ExternalInput").ap()
    # wvt[p, c*512+d] = wv[d, c*128+p]
    wvT = nc.dram_tensor("wvt", [128, KC * 512], fp16, kind="ExternalInput").ap()
    # wot[p, hh*2048+oc] = wo[oc, hh*128+p]
    woT = nc.dram_tensor("wot", [128, GHEADS * HIDDEN], fp16, kind="ExternalInput").ap()
    bq = nc.dram_tensor("bq", [GDIM], fp32, kind="ExternalInput").ap()
    # out[mq, p, oc] = partial[mq*128+p, oc], fp16
    out = nc.dram_tensor("out", [MT, 128, HIDDEN], fp16, kind="ExternalOutput").ap()

    with tile.TileContext(nc) as tc:
        with (
            tc.tile_pool(name="xp", bufs=2) as xp,        # 16KB x-block slots
            tc.tile_pool(name="wqk", bufs=2) as wqk,
            tc.tile_pool(name="wvo", bufs=2) as wvo,
            tc.tile_pool(name="res", bufs=1) as res,
            tc.tile_pool(name="ebp", bufs=1) as ebp,
            tc.tile_pool(name="epp", bufs=2) as epp,
            tc.tile_pool(name="small", bufs=1) as small,
            tc.tile_pool(name="rec", bufs=2) as rec,
            tc.tile_pool(name="sump", bufs=1) as sump,
            tc.tile_pool(name="outp", bufs=2) as outp,
            tc.tile_pool(name="ps_a", bufs=2, space=bass.MemorySpace.PSUM) as ps_a,
            tc.tile_pool(name="ps_sc", bufs=2, space=bass.MemorySpace.PSUM) as ps_sc,
            tc.tile_pool(name="ps_sum", bufs=2, space=bass.MemorySpace.PSUM) as ps_sum,
            tc.tile_pool(name="ps_ctx", bufs=2, space=bass.MemorySpace.PSUM) as ps_ctx,
        ):
            wq_sb = wqk.tile([128, GHEADS * KC * 128], fp16, tag="wqk", name="wq")
            wk_sb = wqk.tile([128, GHEADS * KC * 128], fp16, tag="wqk", name="wk")
            wv_sb = wvo.tile([128, KC * GDIM], fp16, tag="wvo", name="wv")

            qT_sb = res.tile([128, GHEADS * SEQ], fp16, tag="qT")
            kT_sb = res.tile([128, GHEADS * SEQ], fp16, tag="kT")
            v_sb = res.tile([128, MT * GDIM], fp16, tag="v")
            ctx_sb = res.tile([128, GHEADS * SEQ], fp16, tag="ctx")

            eblk = ebp.tile([128, KC * 512], fp16, tag="eblk")

            bq_sb = small.tile([128, GHEADS], fp32, tag="bq")
            ones_sb = small.tile([128, 128], fp16, tag="ones")
            nc.vector.memset(ones_sb[:], 1.0)

            # ---------- P0: initial DMAs + HAM warmup ----------
            # wv + x0 first (v tiles gate P1); x0 split by seq-quarter so
            # v_tile(0,0) starts after ~1.5MB instead of 3MB.
            xv = {0: xp.tile([128, 4 * KC * 128], fp16, tag="xp", name="x0")}
            nc.sync.dma_start(wv_sb[:, 0:4 * GDIM], wvT[:, 0:4 * GDIM])
            for t in range(4):
                nc.sync.dma_start(
                    xv[0][:, t * KC * 128:(t + 1) * KC * 128], xT[0, t])
                if t < 3:
                    nc.sync.dma_start(
                        wv_sb[:, (t + 1) * 4 * GDIM:(t + 2) * 4 * GDIM],
                        wvT[:, (t + 1) * 4 * GDIM:(t + 2) * 4 * GDIM])
            nc.sync.dma_start(bq_sb[:], bq.rearrange("(m p) -> p m", p=128))
            nc.sync.dma_start(wq_sb[:, 0:KC * 128], wqT[0])
            nc.sync.dma_start(wk_sb[:, 0:KC * 128], wkT[0])

            warm = ps_a.tile([128, 512], fp32, tag="ps_a", name="warm")
            for _ in range(48):
                nc.tensor.matmul(warm[:, :128], ones_sb[:], ones_sb[:],
                                 start=True, stop=True)

            def warm_trickle(n):
                wt = ps_ctx.tile([128, 64], fp32, tag="ps_ctx", name="wt")
                for _ in range(n):
                    nc.tensor.matmul(wt[:], ones_sb[:], ones_sb[:, :64],
                                     start=True, stop=True)

            # ---------- helpers ----------
            TQ = KC * 128  # 2048 cols per seq-quarter in an x block

            def v_tile(xblk, st):
                """One [128 seq, 512 dims] v tile (seq tile st = sb*4+t)."""
                t = st % 4
                ps = ps_a.tile([128, 512], fp32, tag="ps_a")
                for c in range(KC):
                    nc.tensor.matmul(
                        ps[:],
                        xblk[:, t * TQ + c * 128: t * TQ + (c + 1) * 128],
                        wv_sb[:, c * GDIM:(c + 1) * GDIM],
                        start=(c == 0), stop=(c == KC - 1))
                nc.vector.tensor_copy(v_sb[:, st * GDIM:(st + 1) * GDIM], ps[:])

            def xmov(xblk, c):
                """Moving-operand AP for chunk c over a 512-seq block:
                [128, 4 quarters, 128] strided view."""
                return xblk[:].rearrange(
                    "p (t r) -> p t r", t=4)[:, :, c * 128:(c + 1) * 128]

            def qk_tile(xblk, w_sb, b_sb, dst, h, s0, nm):
                """One [128 dims, 512 seq] q/k projection tile + bias copy."""
                ps = ps_sc.tile([128, 512], fp32, tag="ps_sc", name=f"pp{nm}")
                hw = h * KC * 128
                for c in range(KC):
                    nc.tensor.matmul(
                        ps[:],
                        w_sb[:, hw + c * 128: hw + (c + 1) * 128],
                        xmov(xblk, c),
                        start=(c == 0), stop=(c == KC - 1))
                nc.scalar.activation(
                    dst[:, h * SEQ + s0: h * SEQ + s0 + 512],
                    ps[:], AF.Identity,
                    bias=(b_sb[:, h:h + 1] if b_sb is not None else 0.0))

            # ---------- P1: v projection + head-0 q/k projection ----------
            for sb in range(SB):
                for t in range(4):
                    v_tile(xv[sb], sb * 4 + t)
                    if sb == 0 and t < 3:
                        warm_trickle(6)
                    if t == 1 and sb + 1 < SB:
                        xv[sb + 1] = xp.tile([128, 4 * TQ], fp16, tag="xp",
                                             name=f"x{sb + 1}")
                        nc.sync.dma_start(
                            xv[sb + 1][:].rearrange("p (t r) -> p t r", t=4),
                            xT[sb + 1].rearrange("t p r -> p t r"))
                    if t == 3 and sb == 0:
                        for h in range(1, GHEADS):
                            nc.sync.dma_start(
                                wq_sb[:, h * KC * 128:(h + 1) * KC * 128],
                                wqT[h])
                            nc.sync.dma_start(
                                wk_sb[:, h * KC * 128:(h + 1) * KC * 128],
                                wkT[h])
                qk_tile(xv[sb], wq_sb, bq_sb, qT_sb, 0, sb * 512, f"q{sb}")
                qk_tile(xv[sb], wk_sb, None, kT_sb, 0, sb * 512, f"k{sb}")

            # ---------- P2/P3: attention windows with PE filler ----------
            state = {}
            pend = []

            def drain(bi, kp):
                h, qb, eblk_, ep, sums, ctxp = state[bi]
                for kc in (2 * kp, 2 * kp + 1):
                    nc.tensor.matmul(ctxp[:],
                                     v_sb[:, kc * GDIM + h * 128:
                                          kc * GDIM + (h + 1) * 128],
                                     eblk_[:, kc * 512:(kc + 1) * 512],
                                     start=(kc == 0), stop=(kc == KC - 1))
                if kp == KC // 2 - 1:
                    if h < GHEADS - 1:
                        # off the critical path: idle GpSimd does the rowsum
                        nc.gpsimd.partition_all_reduce(
                            sums[:], ep[:, 0:512], 128, bass_isa.ReduceOp.add)
                    else:
                        # o_proj filler needs ctx promptly: fast PE rowsum
                        nc.tensor.matmul(sums[:], ones_sb[:],
                                         ep[:, 0:512], start=True, stop=True)
                    finish(bi)

            def finish(bi):
                h, qb, eblk_, ep, sums, ctxp = state.pop(bi)
                q0 = qb * 512
                recip = rec.tile([128, 512], fp32, tag="recip")
                nc.vector.reciprocal_approx_fast(recip[:], sums[:])
                nc.vector.tensor_mul(ctx_sb[:, h * SEQ + q0: h * SEQ + q0 + 512],
                                     ctxp[:], recip[:])

            # filler generators -------------------------------------------
            def proj_filler(h):
                """Yield 128 single-MM closures projecting head h's qT/kT,
                with x re-streamed per seq block (xp slots cycle)."""
                xb = {}

                def load_x(sb):
                    t = xp.tile([128, 4 * TQ], fp16, tag="xp",
                                name=f"xh{h}_{sb}")
                    nc.sync.dma_start(
                        t[:].rearrange("p (t r) -> p t r", t=4),
                        xT[sb].rearrange("t p r -> p t r"))
                    return t

                xb[0] = load_x(0)
                hw = h * KC * 128
                for sb in range(SB):
                    if sb + 1 < SB:
                        xb[sb + 1] = load_x(sb + 1)
                    s0 = sb * 512
                    for w_sb, b_sb, dst, nm in ((wq_sb, bq_sb, qT_sb, "q"),
                                                (wk_sb, None, kT_sb, "k")):
                        ps = ps_a.tile([128, 512], fp32, tag="ps_a",
                                       name=f"p{nm}{h}_{sb}")
                        for c in range(KC):
                            def mm(c=c, ps=ps, w_sb=w_sb, b_sb=b_sb, dst=dst,
                                   sb=sb, s0=s0):
                                nc.tensor.matmul(
                                    ps[:],
                                    w_sb[:, hw + c * 128: hw + (c + 1) * 128],
                                    xmov(xb[sb], c),
                                    start=(c == 0), stop=(c == KC - 1))
                                if c == KC - 1:
                                    nc.scalar.activation(
                                        dst[:, h * SEQ + s0: h * SEQ + s0 + 512],
                                        ps[:], AF.Identity,
                                        bias=(b_sb[:, h:h + 1]
                                              if b_sb is not None else 0.0))
                            yield mm

            def oproj_filler(qb, deep=False, pre=None):
                """Yield single-MM closures for o_proj q-tiles of block qb
                (all heads' ctx for qb must be finished). Tiles in `pre`
                already hold hh=0..2 partials in a PSUM slot from the
                h3-qb0 prefill; only their hh=3 (+copy+DMA) is yielded,
                first, so their banks free before new allocations."""
                pre = pre or {}
                pairs = [(mq, oc) for mq in range(qb * 4, qb * 4 + 4)
                         for oc in range(4)]
                pairs.sort(key=lambda t: t not in pre)
                for i, (mq, oc) in enumerate(pairs):
                    if (mq, oc) in pre:
                        ps = pre[(mq, oc)]
                        ostage = ostages[mq % 2]

                        def mm3(ps=ps, mq=mq, oc=oc, i=i, ostage=ostage):
                            nc.tensor.matmul(
                                ps[:],
                                ctx_sb[:, 3 * SEQ + mq * 128:
                                       3 * SEQ + (mq + 1) * 128],
                                wo_sb[:, 3 * HIDDEN + oc * 512:
                                      3 * HIDDEN + (oc + 1) * 512],
                                start=False, stop=True)
                            dst = ostage[:, oc * 512:(oc + 1) * 512]
                            if i % 2 == 0:
                                nc.vector.tensor_copy(dst, ps[:])
                            else:
                                nc.scalar.activation(dst, ps[:], AF.Copy)
                            nc.sync.dma_start(
                                out[mq][:, oc * 512:(oc + 1) * 512], dst)
                        yield mm3
                        continue
                    if deep and i % 3 == 1:
                        ps = ps_sum.tile([128, 512], fp32, tag="ps_sum",
                                         name=f"po{mq}_{oc}")
                    elif deep and i % 3 == 2:
                        ps = ps_ctx.tile([128, 512], fp32, tag="ps_ctx",
                                         name=f"po{mq}_{oc}")
                    else:
                        ps = ps_a.tile([128, 512], fp32, tag="ps_a",
                                       name=f"po{mq}_{oc}")
                    ostage = ostages[mq % 2]
                    for hh in range(GHEADS):
                        def mm(ps=ps, hh=hh, mq=mq, oc=oc, i=i, ostage=ostage):
                            nc.tensor.matmul(
                                ps[:],
                                ctx_sb[:, hh * SEQ + mq * 128:
                                       hh * SEQ + (mq + 1) * 128],
                                wo_sb[:, hh * HIDDEN + oc * 512:
                                      hh * HIDDEN + (oc + 1) * 512],
                                start=(hh == 0), stop=(hh == GHEADS - 1))
                            if hh == GHEADS - 1:
                                dst = ostage[:, oc * 512:(oc + 1) * 512]
                                if i % 2 == 0:
                                    nc.vector.tensor_copy(dst, ps[:])
                                else:
                                    nc.scalar.activation(dst, ps[:], AF.Copy)
                                nc.sync.dma_start(
                                    out[mq][:, oc * 512:(oc + 1) * 512], dst)
                        yield mm

            ostages = [outp.tile([128, HIDDEN], fp16, tag="out", name=f"os{i}")
                       for i in range(2)]

            bi = 0
            for h in range(GHEADS):
                if h == GHEADS - 2:
                    # prefetch wo a full head early: the h3-qb0 o_proj
                    # prefill must not stall on this 2MB load
                    wo_sb = wvo.tile([128, GHEADS * HIDDEN], fp16, tag="wvo",
                                     name="wo")
                    nc.sync.dma_start(wo_sb[:], woT)
                if h < GHEADS - 1:
                    filler = proj_filler(h + 1)
                    per_stage = 4
                else:
                    filler = None  # switched per q-block below
                    per_stage = 8
                    pre_state = {}
                    pre_mms = []
                    # h3-qb0 is ACT-bound (no filler available): prefill
                    # hh=0..2 o_proj partials for qb0 into idle PSUM slots
                    for (mq, oc, pool, ptag) in ((0, 0, ps_a, "ps_a"),
                                                 (0, 1, ps_a, "ps_a"),
                                                 (0, 2, ps_sum, "ps_sum")):
                        ps = pool.tile([128, 512], fp32, tag=ptag,
                                       name=f"pre{mq}_{oc}")
                        pre_state[(mq, oc)] = ps
                        for hh in range(3):
                            def mm(ps=ps, hh=hh, mq=mq, oc=oc):
                                nc.tensor.matmul(
                                    ps[:],
                                    ctx_sb[:, hh * SEQ + mq * 128:
                                           hh * SEQ + (mq + 1) * 128],
                                    wo_sb[:, hh * HIDDEN + oc * 512:
                                          hh * HIDDEN + (oc + 1) * 512],
                                    start=(hh == 0), stop=False)
                            pre_mms.append(mm)
                for qb in range(QB):
                    if h == GHEADS - 1 and qb >= 1:
                        filler = oproj_filler(qb - 1, pre=pre_state)
                        pre_state = {}
                    hq = h * SEQ
                    q0 = qb * 512
                    ep = epp.tile([128, KC // 2 * 512], fp16, tag="ep")
                    if h < GHEADS - 1:
                        sums = sump.tile([128, 512], fp32, tag="sums")
                    else:
                        sums = ps_sum.tile([128, 512], fp32, tag="ps_sum")
                    ctxp = ps_ctx.tile([128, 512], fp32, tag="ps_ctx")
                    state[bi] = (h, qb, eblk, ep, sums, ctxp)
                    for kp in range(KC // 2):
                        for i in (0, 1):
                            kc = 2 * kp + i
                            sc = ps_sc.tile([128, 512], fp32, tag="ps_sc")
                            nc.tensor.matmul(
                                sc[:],
                                kT_sb[:, hq + kc * 128: hq + (kc + 1) * 128],
                                qT_sb[:, hq + q0: hq + q0 + 512],
                                start=True, stop=True)
                            nc.scalar.activation(
                                eblk[:, kc * 512:(kc + 1) * 512], sc[:], AF.Exp)
                        nc.vector.tensor_add(
                            ep[:, kp * 512:(kp + 1) * 512],
                            eblk[:, (2 * kp) * 512:(2 * kp + 1) * 512],
                            eblk[:, (2 * kp + 1) * 512:(2 * kp + 2) * 512])
                        if kp % 2 == 1:
                            nc.vector.tensor_add(
                                ep[:, (kp - 1) * 512: kp * 512],
                                ep[:, (kp - 1) * 512: kp * 512],
                                ep[:, kp * 512:(kp + 1) * 512])
                        if kp % 4 == 3:
                            nc.vector.tensor_add(
                                ep[:, (kp - 3) * 512:(kp - 2) * 512],
                                ep[:, (kp - 3) * 512:(kp - 2) * 512],
                                ep[:, (kp - 1) * 512: kp * 512])
                        if kp == KC // 2 - 1:
                            nc.vector.tensor_add(
                                ep[:, 0:512], ep[:, 0:512],
                                ep[:, 4 * 512:5 * 512])
                        for b_kp in pend:
                            drain(*b_kp)
                        pend = [(bi, kp)]
                        if filler is not None:
                            for _ in range(per_stage):
                                mm = next(filler, None)
                                if mm is not None:
                                    mm()
                        elif h == GHEADS - 1 and pre_mms:
                            for mm in pre_mms[:2]:
                                mm()
                            del pre_mms[:2]
                    if h == GHEADS - 1 and filler is not None:
                        for mm in filler:  # defensive: never drop filler work
                            mm()
                    bi += 1
                if h < GHEADS - 1 and filler is not None:
                    for mm in filler:
                        mm()
            for b_kp in pend:
                drain(*b_kp)

            # ---------- P4: leftover o_proj (last q-block) ----------
            # interleave tiles in groups of 3 (one per PSUM pool), with each
            # tile's hh=3 matmul deferred so the last ctx normalize (recip +
            # mul on VectorE) is off the PE critical path
            p4 = list(oproj_filler(QB - 1, deep=True))
            order = []
            for g in range(0, 16, 3):
                tiles = [p4[t * 4:(t + 1) * 4] for t in range(g, min(g + 3, 16))]
                for tl in tiles:
                    order += tl[:3]
                for tl in tiles:
                    order.append(tl[3])
            for mm in order:
                mm()

    nc.compile()
    return nc


def kernel(x, wq, bq, wk, bk, wv, bv, wo, bo):
    from concourse import bass_utils

    if "nc" not in _CACHE:
        _CACHE["nc"] = _build()
    nc = _CACHE["nc"]

    x = np.asarray(x, np.float32)
    wq = np.asarray(wq, np.float32)
    wk = np.asarray(wk, np.float32)
    wv = np.asarray(wv, np.float32)
    wo = np.asarray(wo, np.float32)
    scale = np.float32(1.0 / np.sqrt(HEAD_DIM))

    # xt[sb, t, p, c*128+s'] = x[b, sb*512+t*128+s', c*128+p]
    xT = [np.ascontiguousarray(
        x[b].reshape(SB, 4, 128, KC, 128).transpose(0, 1, 4, 3, 2)
        .reshape(SB, 4, 128, KC * 128)).astype(np.float16) for b in range(BATCH)]

    in_maps = []
    for j in range(N_CORES):
        b, g = divmod(j, GROUPS)
        ds = slice(g * GDIM, (g + 1) * GDIM)
        wq_s = (wq[ds] * scale).reshape(GHEADS, 128, KC, 128).transpose(0, 3, 2, 1)
        wk_s = wk[ds].reshape(GHEADS, 128, KC, 128).transpose(0, 3, 2, 1)
        wv_s = wv[ds].reshape(GDIM, KC, 128).transpose(2, 1, 0)
        wo_s = wo[:, ds].T.reshape(GHEADS, 128, HIDDEN).transpose(1, 0, 2)
        in_maps.append({
            "xt": xT[b],
            "wqt": np.ascontiguousarray(
                wq_s.reshape(GHEADS, 128, KC * 128)).astype(np.float16),
            "wkt": np.ascontiguousarray(
                wk_s.reshape(GHEADS, 128, KC * 128)).astype(np.float16),
            "wvt": np.ascontiguousarray(
                wv_s.reshape(128, KC * GDIM)).astype(np.float16),
            "wot": np.ascontiguousarray(
                wo_s.reshape(128, GHEADS * HIDDEN)).astype(np.float16),
            "bq": (np.asarray(bq)[ds] * scale).astype(np.float32),
        })

    res = bass_utils.run_bass_kernel_spmd(
        nc, in_maps, core_ids=list(range(N_CORES)),
        **_CACHE.get("run_kwargs", {}))
    _CACHE["last_res"] = res

    outp = np.zeros((BATCH, MT, 128, HIDDEN), np.float32)
    for j in range(N_CORES):
        b = j // GROUPS
        outp[b] += res.results[j]["out"].astype(np.float32)
    outp = outp.reshape(BATCH, SEQ, HIDDEN)
    bo_eff = np.asarray(bo, np.float32) + wo @ np.asarray(bv, np.float32)
    return outp + bo_eff


# revision 17
# speedup vs baseline: 1.0145x; 1.0025x over previous
"""Multi-head attention (B=2, S=2048, H=2048, 16 heads, d=128) on 8 TRN2
NeuronCores.

Sharding: 2-way batch x 4-way head-group tensor parallel. Core j handles
batch j//4 and heads 4*(j%4)..4*(j%4)+3 (a 512-wide slice of the qkv
projection output dim / o_proj input dim). Each core returns a partial
o_proj output [S, H] in fp16; the host sums the 4 partials per batch and
adds an effective bias bo + wo@bv (softmax rows sum to 1, so the v bias
contributes a constant; bk cancels inside softmax and is dropped).

All fp16 matmul operands, fp32 PSUM accumulation. DRAM layouts keep
16KB-contiguous per-partition rows (~420GB/s DMA vs ~200 for 1KB rows);
x block 0 is split into 4 seq-quarter DMAs so the first v tile starts
~5us earlier. One interleaved PE stream:

  P1: v = x@wv.T and head 0's qT/kT projections, seq-blocked, while x
      streams in.
  P2: for h in 0..2: attention(h) stages with head h+1's qT/kT projection
      matmuls as per-stage PE filler (x re-streamed from HBM per head).
  P3: attention(h=3) with o_proj matmuls of completed q-blocks as filler.
  P4: leftover o_proj through all idle PSUM banks.

Attention per (head, q-block), scoresT layout [k, q]:
  scoresT = kT_h.T @ qT_h -> exp on ScalarE -> expT (fp16)
  VectorE pair-adds exp chunks; sums += ones128.T @ pair  (half-cost rowsum)
  ctxT += v_chunk.T @ expT ; ctxT_norm = ctxT * approx_recip(sums)
"""
import sys

if "/opt/trn_rl_repo" not in sys.path:
    sys.path.insert(0, "/opt/trn_rl_repo")

import numpy as np

HIDDEN = 2048
HEADS = 16
HEAD_DIM = 128
BATCH = 2
SEQ = 2048

N_CORES = 8
GROUPS = 4               # head groups (cores per batch)
GDIM = HIDDEN // GROUPS  # 512 dims per core
GHEADS = GDIM // HEAD_DIM  # 4 heads per core
KC = HIDDEN // 128       # 16 contraction chunks
SB = 4                   # seq blocks of 512
QB = SEQ // 512          # 4 q-blocks in attention
MT = SEQ // 128          # 16 seq tiles of 128

_CACHE = {}


def _build():
    import concourse.bacc as bacc
    import concourse.bass as bass
    import concourse.bass_isa as bass_isa
    import concourse.mybir as mybir
    import concourse.tile as tile

    fp16 = mybir.dt.float16
    fp32 = mybir.dt.float32
    AF = mybir.ActivationFunctionType

    nc = bacc.Bacc("TRN2", target_bir_lowering=False, debug=False,
                   num_devices=N_CORES)

    # xt[sb, t, p, c*128+s'] = x[b, sb*512+t*128+s', c*128+p]
    xT = nc.dram_tensor("xt", [SB, 4, 128, KC * 128], fp16, kind="ExternalInput").ap()
    # wqt[h, p, c*128+m] = wq_scaled[h*128+m, c*128+p]
    wqT = nc.dram_tensor("wqt", [GHEADS, 128, KC * 128], fp16, kind="ExternalInput").ap()
    wkT = nc.dram_tensor("wkt", [GHEADS, 128, KC * 128], fp16, kind="ExternalInput").ap()
    # wvt[p, c*512+d] = wv[d, c*128+p]
    wvT = nc.dram_tensor("wvt", [128, KC * 512], fp16, kind="ExternalInput").ap()
    # wot[p, hh*2048+oc] = wo[oc, hh*128+p]
    woT = nc.dram_tensor("wot", [128, GHEADS * HIDDEN], fp16, kind="ExternalInput").ap()
    bq = nc.dram_tensor("bq", [GDIM], fp32, kind="ExternalInput").ap()
    # out[mq, p, oc] = partial[mq*128+p, oc], fp16
    out = nc.dram_tensor("out", [MT, 128, HIDDEN], fp16, kind="ExternalOutput").ap()

    with tile.TileContext(nc) as tc:
        with (
            tc.tile_pool(name="xp", bufs=2) as xp,        # 16KB x-block slots
            tc.tile_pool(name="wqk", bufs=2) as wqk,
            tc.tile_pool(name="wvo", bufs=2) as wvo,
            tc.tile_pool(name="res", bufs=1) as res,
            tc.tile_pool(name="ebp", bufs=1) as ebp,
            tc.tile_pool(name="epp", bufs=2) as epp,
            tc.tile_pool(name="small", bufs=1) as small,
            tc.tile_pool(name="rec", bufs=2) as rec,
            tc.tile_pool(name="sump", bufs=1) as sump,
            tc.tile_pool(name="park", bufs=1) as parkp,
            tc.tile_pool(name="outp", bufs=2) as outp,
            tc.tile_pool(name="ps_a", bufs=2, space=bass.MemorySpace.PSUM) as ps_a,
            tc.tile_pool(name="ps_sc", bufs=2, space=bass.MemorySpace.PSUM) as ps_sc,
            tc.tile_pool(name="ps_sum", bufs=2, space=bass.MemorySpace.PSUM) as ps_sum,
            tc.tile_pool(name="ps_ctx", bufs=2, space=bass.MemorySpace.PSUM) as ps_ctx,
        ):
            wq_sb = wqk.tile([128, GHEADS * KC * 128], fp16, tag="wqk", name="wq")
            wk_sb = wqk.tile([128, GHEADS * KC * 128], fp16, tag="wqk", name="wk")
            wv_sb = wvo.tile([128, KC * GDIM], fp16, tag="wvo", name="wv")

            qT_sb = res.tile([128, GHEADS * SEQ], fp16, tag="qT")
            kT_sb = res.tile([128, GHEADS * SEQ], fp16, tag="kT")
            v_sb = res.tile([128, MT * GDIM], fp16, tag="v")
            ctx_sb = res.tile([128, GHEADS * SEQ], fp16, tag="ctx")

            eblk = ebp.tile([128, KC * 512], fp16, tag="eblk")

            bq_sb = small.tile([128, GHEADS], fp32, tag="bq")
            ones_sb = small.tile([128, 128], fp16, tag="ones")
            nc.vector.memset(ones_sb[:], 1.0)

            # ---------- P0: initial DMAs + HAM warmup ----------
            # wv + x0 first (v tiles gate P1); x0 split by seq-quarter so
            # v_tile(0,0) starts after ~1.5MB instead of 3MB.
            xv = {0: xp.tile([128, 4 * KC * 128], fp16, tag="xp", name="x0")}
            nc.sync.dma_start(wv_sb[:, 0:4 * GDIM], wvT[:, 0:4 * GDIM])
            for t in range(4):
                nc.sync.dma_start(
                    xv[0][:, t * KC * 128:(t + 1) * KC * 128], xT[0, t])
                if t < 3:
                    nc.sync.dma_start(
                        wv_sb[:, (t + 1) * 4 * GDIM:(t + 2) * 4 * GDIM],
                        wvT[:, (t + 1) * 4 * GDIM:(t + 2) * 4 * GDIM])
            nc.sync.dma_start(bq_sb[:], bq.rearrange("(m p) -> p m", p=128))
            nc.sync.dma_start(wq_sb[:, 0:KC * 128], wqT[0])
            nc.sync.dma_start(wk_sb[:, 0:KC * 128], wkT[0])

            warm = ps_a.tile([128, 512], fp32, tag="ps_a", name="warm")
            for _ in range(48):
                nc.tensor.matmul(warm[:, :128], ones_sb[:], ones_sb[:],
                                 start=True, stop=True)

            def warm_trickle(n):
                wt = ps_ctx.tile([128, 64], fp32, tag="ps_ctx", name="wt")
                for _ in range(n):
                    nc.tensor.matmul(wt[:], ones_sb[:], ones_sb[:, :64],
                                     start=True, stop=True)

            # ---------- helpers ----------
            TQ = KC * 128  # 2048 cols per seq-quarter in an x block

            def v_tile(xblk, st):
                """One [128 seq, 512 dims] v tile (seq tile st = sb*4+t)."""
                t = st % 4
                ps = ps_a.tile([128, 512], fp32, tag="ps_a")
                for c in range(KC):
                    nc.tensor.matmul(
                        ps[:],
                        xblk[:, t * TQ + c * 128: t * TQ + (c + 1) * 128],
                        wv_sb[:, c * GDIM:(c + 1) * GDIM],
                        start=(c == 0), stop=(c == KC - 1))
                nc.vector.tensor_copy(v_sb[:, st * GDIM:(st + 1) * GDIM], ps[:])

            def xmov(xblk, c):
                """Moving-operand AP for chunk c over a 512-seq block:
                [128, 4 quarters, 128] strided view."""
                return xblk[:].rearrange(
                    "p (t r) -> p t r", t=4)[:, :, c * 128:(c + 1) * 128]

            def qk_tile(xblk, w_sb, b_sb, dst, h, s0, nm):
                """One [128 dims, 512 seq] q/k projection tile + bias copy."""
                ps = ps_sc.tile([128, 512], fp32, tag="ps_sc", name=f"pp{nm}")
                hw = h * KC * 128
                for c in range(KC):
                    nc.tensor.matmul(
                        ps[:],
                        w_sb[:, hw + c * 128: hw + (c + 1) * 128],
                        xmov(xblk, c),
                        start=(c == 0), stop=(c == KC - 1))
                nc.scalar.activation(
                    dst[:, h * SEQ + s0: h * SEQ + s0 + 512],
                    ps[:], AF.Identity,
                    bias=(b_sb[:, h:h + 1] if b_sb is not None else 0.0))

            # ---------- P1: v projection + head-0 q/k projection ----------
            for sb in range(SB):
                for t in range(4):
                    v_tile(xv[sb], sb * 4 + t)
                    if sb == 0 and t < 3:
                        warm_trickle(6)
                    if t == 1 and sb + 1 < SB:
                        xv[sb + 1] = xp.tile([128, 4 * TQ], fp16, tag="xp",
                                             name=f"x{sb + 1}")
                        nc.sync.dma_start(
                            xv[sb + 1][:].rearrange("p (t r) -> p t r", t=4),
                            xT[sb + 1].rearrange("t p r -> p t r"))
                    if t == 3 and sb == 0:
                        for h in range(1, GHEADS):
                            nc.sync.dma_start(
                                wq_sb[:, h * KC * 128:(h + 1) * KC * 128],
                                wqT[h])
                            nc.sync.dma_start(
                                wk_sb[:, h * KC * 128:(h + 1) * KC * 128],
                                wkT[h])
                qk_tile(xv[sb], wq_sb, bq_sb, qT_sb, 0, sb * 512, f"q{sb}")
                qk_tile(xv[sb], wk_sb, None, kT_sb, 0, sb * 512, f"k{sb}")

            # ---------- P2/P3: attention windows with PE filler ----------
            state = {}
            pend = []

            def drain(bi, kp):
                h, qb, eblk_, ep, sums, ctxp = state[bi]
                for kc in (2 * kp, 2 * kp + 1):
                    nc.tensor.matmul(ctxp[:],
                                     v_sb[:, kc * GDIM + h * 128:
                                          kc * GDIM + (h + 1) * 128],
                                     eblk_[:, kc * 512:(kc + 1) * 512],
                                     start=(kc == 0), stop=(kc == KC - 1))
                if kp == KC // 2 - 1:
                    if h < GHEADS - 1:
                        # off the critical path: idle GpSimd does the rowsum
                        nc.gpsimd.partition_all_reduce(
                            sums[:], ep[:, 0:512], 128, bass_isa.ReduceOp.add)
                    else:
                        # o_proj filler needs ctx promptly: fast PE rowsum
                        nc.tensor.matmul(sums[:], ones_sb[:],
                                         ep[:, 0:512], start=True, stop=True)
                    finish(bi)

            def finish(bi):
                h, qb, eblk_, ep, sums, ctxp = state.pop(bi)
                q0 = qb * 512
                recip = rec.tile([128, 512], fp32, tag="recip")
                nc.vector.reciprocal_approx_fast(recip[:], sums[:])
                nc.vector.tensor_mul(ctx_sb[:, h * SEQ + q0: h * SEQ + q0 + 512],
                                     ctxp[:], recip[:])

            # filler generators -------------------------------------------
            def proj_filler(h):
                """Yield 128 single-MM closures projecting head h's qT/kT,
                with x re-streamed per seq block (xp slots cycle)."""
                xb = {}

                def load_x(sb):
                    t = xp.tile([128, 4 * TQ], fp16, tag="xp",
                                name=f"xh{h}_{sb}")
                    nc.sync.dma_start(
                        t[:].rearrange("p (t r) -> p t r", t=4),
                        xT[sb].rearrange("t p r -> p t r"))
                    return t

                xb[0] = load_x(0)
                hw = h * KC * 128
                for sb in range(SB):
                    if sb + 1 < SB:
                        xb[sb + 1] = load_x(sb + 1)
                    s0 = sb * 512
                    for w_sb, b_sb, dst, nm in ((wq_sb, bq_sb, qT_sb, "q"),
                                                (wk_sb, None, kT_sb, "k")):
                        ps = ps_a.tile([128, 512], fp32, tag="ps_a",
                                       name=f"p{nm}{h}_{sb}")
                        for c in range(KC):
                            def mm(c=c, ps=ps, w_sb=w_sb, b_sb=b_sb, dst=dst,
                                   sb=sb, s0=s0):
                                nc.tensor.matmul(
                                    ps[:],
                                    w_sb[:, hw + c * 128: hw + (c + 1) * 128],
                                    xmov(xb[sb], c),
                                    start=(c == 0), stop=(c == KC - 1))
                                if c == KC - 1:
                                    nc.scalar.activation(
                                        dst[:, h * SEQ + s0: h * SEQ + s0 + 512],
                                        ps[:], AF.Identity,
                                        bias=(b_sb[:, h:h + 1]
                                              if b_sb is not None else 0.0))
                            yield mm

            def oproj_filler(qb, deep=False, pre=None):
                """Yield single-MM closures for o_proj q-tiles of block qb
                (all heads' ctx for qb must be finished). Tiles in `pre`
                already hold hh=0..2 partials in a PSUM slot from the
                h3-qb0 prefill; only their hh=3 (+copy+DMA) is yielded,
                first, so their banks free before new allocations."""
                pre = pre or {}
                rank = {(0, 1): 0, (0, 3): 1, (0, 2): 2, (0, 0): 3}
                pairs = [(mq, oc) for mq in range(qb * 4, qb * 4 + 4)
                         for oc in range(4)]
                pairs.sort(key=lambda t: (t not in pre, rank.get(t, 9)))
                for i, (mq, oc) in enumerate(pairs):
                    if (mq, oc) in pre:
                        ps = pre[(mq, oc)]
                        ostage = ostages[mq % 2]
                        if isinstance(ps, tuple):  # parked fp16 partial
                            park = ps[1]

                            def mmp(park=park, mq=mq, oc=oc, ostage=ostage):
                                psn = ps_a.tile([128, 512], fp32, tag="ps_a",
                                                name="prepark")
                                nc.tensor.matmul(
                                    psn[:],
                                    ctx_sb[:, 3 * SEQ + mq * 128:
                                           3 * SEQ + (mq + 1) * 128],
                                    wo_sb[:, 3 * HIDDEN + oc * 512:
                                          3 * HIDDEN + (oc + 1) * 512],
                                    start=True, stop=True)
                                dst = ostage[:, oc * 512:(oc + 1) * 512]
                                nc.vector.tensor_add(dst, park[:], psn[:])
                                nc.sync.dma_start(
                                    out[mq][:, oc * 512:(oc + 1) * 512], dst)
                            yield mmp
                            continue

                        def mm3(ps=ps, mq=mq, oc=oc, i=i, ostage=ostage):
                            nc.tensor.matmul(
                                ps[:],
                                ctx_sb[:, 3 * SEQ + mq * 128:
                                       3 * SEQ + (mq + 1) * 128],
                                wo_sb[:, 3 * HIDDEN + oc * 512:
                                      3 * HIDDEN + (oc + 1) * 512],
                                start=False, stop=True)
                            dst = ostage[:, oc * 512:(oc + 1) * 512]
                            if i % 2 == 0:
                                nc.vector.tensor_copy(dst, ps[:])
                            else:
                                nc.scalar.activation(dst, ps[:], AF.Copy)
                            nc.sync.dma_start(
                                out[mq][:, oc * 512:(oc + 1) * 512], dst)
                        yield mm3
                        continue
                    if deep and i % 3 == 1:
                        ps = ps_sum.tile([128, 512], fp32, tag="ps_sum",
                                         name=f"po{mq}_{oc}")
                    elif deep and i % 3 == 2:
                        ps = ps_ctx.tile([128, 512], fp32, tag="ps_ctx",
                                         name=f"po{mq}_{oc}")
                    else:
                        ps = ps_a.tile([128, 512], fp32, tag="ps_a",
                                       name=f"po{mq}_{oc}")
                    ostage = ostages[mq % 2]
                    for hh in range(GHEADS):
                        def mm(ps=ps, hh=hh, mq=mq, oc=oc, i=i, ostage=ostage):
                            nc.tensor.matmul(
                                ps[:],
                                ctx_sb[:, hh * SEQ + mq * 128:
                                       hh * SEQ + (mq + 1) * 128],
                                wo_sb[:, hh * HIDDEN + oc * 512:
                                      hh * HIDDEN + (oc + 1) * 512],
                                start=(hh == 0), stop=(hh == GHEADS - 1))
                            if hh == GHEADS - 1:
                                dst = ostage[:, oc * 512:(oc + 1) * 512]
                                if i % 2 == 0:
                                    nc.vector.tensor_copy(dst, ps[:])
                                else:
                                    nc.scalar.activation(dst, ps[:], AF.Copy)
                                nc.sync.dma_start(
                                    out[mq][:, oc * 512:(oc + 1) * 512], dst)
                        yield mm

            ostages = [outp.tile([128, HIDDEN], fp16, tag="out", name=f"os{i}")
                       for i in range(2)]

            bi = 0
            for h in range(GHEADS):
                if h == GHEADS - 2:
                    # prefetch wo a full head early: the h3-qb0 o_proj
                    # prefill must not stall on this 2MB load
                    wo_sb = wvo.tile([128, GHEADS * HIDDEN], fp16, tag="wvo",
                                     name="wo")
                    nc.sync.dma_start(wo_sb[:], woT)
                if h < GHEADS - 1:
                    filler = proj_filler(h + 1)
                    per_stage = 4
                else:
                    filler = None  # switched per q-block below
                    per_stage = 8
                    pre_state = {}
                    pre_mms = []
                    park_sb = parkp.tile([128, 512], fp16, tag="park")

                    # h3-qb0 is ACT-bound (no filler available): prefill
                    # hh=0..2 o_proj partials for qb0 into idle PSUM slots.
                    # Tile (0,0) parks its partial to fp16 SBUF mid-hole so
                    # its bank can host tile (0,3)'s partial too.
                    def pre_tile(mq, oc, pool, ptag, stop=False):
                        ps = pool.tile([128, 512], fp32, tag=ptag,
                                       name=f"pre{mq}_{oc}")
                        for hh in range(3):
                            def mm(ps=ps, hh=hh, mq=mq, oc=oc):
                                nc.tensor.matmul(
                                    ps[:],
                                    ctx_sb[:, hh * SEQ + mq * 128:
                                           hh * SEQ + (mq + 1) * 128],
                                    wo_sb[:, hh * HIDDEN + oc * 512:
                                          hh * HIDDEN + (oc + 1) * 512],
                                    start=(hh == 0), stop=(stop and hh == 2))
                            pre_mms.append(mm)
                        return ps

                    # consume order (encoded below): (0,1),(0,3),(0,2),(0,0)
                    ps00 = pre_tile(0, 0, ps_a, "ps_a", stop=True)
                    pre_mms.append(lambda: nc.vector.tensor_copy(
                        park_sb[:], ps00[:]))
                    pre_state[(0, 1)] = pre_tile(0, 1, ps_a, "ps_a")
                    pre_state[(0, 2)] = pre_tile(0, 2, ps_sum, "ps_sum")
                    pre_state[(0, 3)] = pre_tile(0, 3, ps_a, "ps_a")
                    pre_state[(0, 0)] = ("park", park_sb)
                for qb in range(QB):
                    if h == GHEADS - 1 and qb >= 1:
                        filler = oproj_filler(qb - 1, pre=pre_state)
                        pre_state = {}
                    hq = h * SEQ
                    q0 = qb * 512
                    ep = epp.tile([128, KC // 2 * 512], fp16, tag="ep")
                    if h < GHEADS - 1:
                        sums = sump.tile([128, 512], fp32, tag="sums")
                    else:
                        sums = ps_sum.tile([128, 512], fp32, tag="ps_sum")
                    ctxp = ps_ctx.tile([128, 512], fp32, tag="ps_ctx")
                    state[bi] = (h, qb, eblk, ep, sums, ctxp)
                    for kp in range(KC // 2):
                        for i in (0, 1):
                            kc = 2 * kp + i
                            sc = ps_sc.tile([128, 512], fp32, tag="ps_sc")
                            nc.tensor.matmul(
                                sc[:],
                                kT_sb[:, hq + kc * 128: hq + (kc + 1) * 128],
                                qT_sb[:, hq + q0: hq + q0 + 512],
                                start=True, stop=True)
                            nc.scalar.activation(
                                eblk[:, kc * 512:(kc + 1) * 512], sc[:], AF.Exp)
                        nc.vector.tensor_add(
                            ep[:, kp * 512:(kp + 1) * 512],
                            eblk[:, (2 * kp) * 512:(2 * kp + 1) * 512],
                            eblk[:, (2 * kp + 1) * 512:(2 * kp + 2) * 512])
                        if kp % 2 == 1:
                            nc.vector.tensor_add(
                                ep[:, (kp - 1) * 512: kp * 512],
                                ep[:, (kp - 1) * 512: kp * 512],
                                ep[:, kp * 512:(kp + 1) * 512])
                        if kp % 4 == 3:
                            nc.vector.tensor_add(
                                ep[:, (kp - 3) * 512:(kp - 2) * 512],
                                ep[:, (kp - 3) * 512:(kp - 2) * 512],
                                ep[:, (kp - 1) * 512: kp * 512])
                        if kp == KC // 2 - 1:
                            nc.vector.tensor_add(
                                ep[:, 0:512], ep[:, 0:512],
                                ep[:, 4 * 512:5 * 512])
                        for b_kp in pend:
                            drain(*b_kp)
                        pend = [(bi, kp)]
                        if filler is not None:
                            for _ in range(per_stage):
                                mm = next(filler, None)
                                if mm is not None:
                                    mm()
                        elif h == GHEADS - 1 and pre_mms:
                            for mm in pre_mms[:2]:
                                mm()
                            del pre_mms[:2]
                    if h == GHEADS - 1 and filler is not None:
                        for mm in filler:  # defensive: never drop filler work
                            mm()
                    bi += 1
                if h < GHEADS - 1 and filler is not None:
                    for mm in filler:
                        mm()
            for b_kp in pend:
                drain(*b_kp)

            # ---------- P4: leftover o_proj (last q-block) ----------
            # interleave tiles in groups of 3 (one per PSUM pool), with each
            # tile's hh=3 matmul deferred so the last ctx normalize (recip +
            # mul on VectorE) is off the PE critical path
            p4 = list(oproj_filler(QB - 1, deep=True))
            order = []
            for g in range(0, 16, 3):
                tiles = [p4[t * 4:(t + 1) * 4] for t in range(g, min(g + 3, 16))]
                for tl in tiles:
                    order += tl[:3]
                for tl in tiles:
                    order.append(tl[3])
            for mm in order:
                mm()

    nc.compile()
    return nc


def kernel(x, wq, bq, wk, bk, wv, bv, wo, bo):
    from concourse import bass_utils

    if "nc" not in _CACHE:
        _CACHE["nc"] = _build()
    nc = _CACHE["nc"]

    x = np.asarray(x, np.float32)
    wq = np.asarray(wq, np.float32)
    wk = np.asarray(wk, np.float32)
    wv = np.asarray(wv, np.float32)
    wo = np.asarray(wo, np.float32)
    scale = np.float32(1.0 / np.sqrt(HEAD_DIM))

    # xt[sb, t, p, c*128+s'] = x[b, sb*512+t*128+s', c*128+p]
    xT = [np.ascontiguousarray(
        x[b].reshape(SB, 4, 128, KC, 128).transpose(0, 1, 4, 3, 2)
        .reshape(SB, 4, 128, KC * 128)).astype(np.float16) for b in range(BATCH)]

    in_maps = []
    for j in range(N_CORES):
        b, g = divmod(j, GROUPS)
        ds = slice(g * GDIM, (g + 1) * GDIM)
        wq_s = (wq[ds] * scale).reshape(GHEADS, 128, KC, 128).transpose(0, 3, 2, 1)
        wk_s = wk[ds].reshape(GHEADS, 128, KC, 128).transpose(0, 3, 2, 1)
        wv_s = wv[ds].reshape(GDIM, KC, 128).transpose(2, 1, 0)
        wo_s = wo[:, ds].T.reshape(GHEADS, 128, HIDDEN).transpose(1, 0, 2)
        in_maps.append({
            "xt": xT[b],
            "wqt": np.ascontiguousarray(
                wq_s.reshape(GHEADS, 128, KC * 128)).astype(np.float16),
            "wkt": np.ascontiguousarray(
                wk_s.reshape(GHEADS, 128, KC * 128)).astype(np.float16),
            "wvt": np.ascontiguousarray(
                wv_s.reshape(128, KC * GDIM)).astype(np.float16),
            "wot": np.ascontiguousarray(
                wo_s.reshape(128, GHEADS * HIDDEN)).astype(np.float16),
            "bq": (np.asarray(bq)[ds] * scale).astype(np.float32),
        })

    res = bass_utils.run_bass_kernel_spmd(
        nc, in_maps, core_ids=list(range(N_CORES)),
        **_CACHE.get("run_kwargs", {}))
    _CACHE["last_res"] = res

    outp = np.zeros((BATCH, MT, 128, HIDDEN), np.float32)
    for j in range(N_CORES):
        b = j // GROUPS
        outp[b] += res.results[j]["out"].astype(np.float32)
    outp = outp.reshape(BATCH, SEQ, HIDDEN)
    bo_eff = np.asarray(bo, np.float32) + wo @ np.asarray(bv, np.float32)
    return outp + bo_eff
